# revision 36
# baseline (speedup 1.0000x reference)
"""Trainium2 Bass kernel for nn_Attention_10015863734775.

Multi-head causal attention (16 heads, d_model 2048, d_head 128, seq 2048,
batch 1) with llama-style interleaved RoPE and a signed-softmax:
    attn_w = sign(s) * exp(|s| - max|s|);  attn = attn_w / (sum|attn_w| + 1e-6)
The max-subtraction cancels in the normalization (scores are O(5), exp is
safe in fp32), so the device computes attn = sign(s)exp(|s|) / sum exp(|s|).

Sharding: 2 heads per NeuronCore (8 cores). Each core receives the full
transposed residual X^T plus its head slices of W_Q/K/V/W_O and computes a
partial output projection outT_c[m, s] (bf16); the host sums the 8 partials
in f32, transposes, and adds b_O (exact: b_O enters after all nonlinearities).

Device layouts are all [feature, seq] ("T" layouts) so that:
  - scoresT[k, q] blocks come straight from matmul(lhsT=kT chunk, rhs=qT)
  - the z matmul needs no transposes (V is transposed once via the PE)
  - the signed-softmax k-sum is a ones-vector matmul on the PE
Causal masking skips fully-masked blocks; diagonal blocks add a -1e5 mask
to |s| before exp (exp underflows to exactly 0).
All matmuls run as float32r (~tf32 operand rounding, full fp32 accumulate).

Deltas over the original baseline (each measured):
  - reciprocal -> reciprocal_approx_fast (~5x) + f32r rounding copy
  - output staging + DMA in bf16 (halves output traffic; host sums in f32)
  - causal-mask adds on GPSIMD (SBUF-only op; frees DVE/ACT cycles)
"""

import math

import numpy as np

S = 2048          # sequence length
D = 2048          # d_model
DH = 128          # d_head
NH = 16           # total heads
NC = 8            # neuron cores
HPC = NH // NC    # heads per core (2)
ST = 512          # seq tile (matmul free dim / one PSUM bank)
NST = S // ST     # 4 seq tiles
NDC = D // 128    # 16 contraction chunks
NKC = S // 128    # 16 k chunks
C_SCALE = 1.0 / math.sqrt(float(DH))
LN2 = math.log(2.0)
MASK_NEG = -1.0e5

_CACHE = {}


def _build_program():
    import concourse.tile as tile
    from concourse import bacc, mybir

    F32 = mybir.dt.float32
    F32R = mybir.dt.float32r
    BF16 = mybir.dt.bfloat16
    AF = mybir.ActivationFunctionType
    ALU = mybir.AluOpType

    nc = bacc.Bacc("TRN2", target_bir_lowering=False, debug=False, num_devices=NC)

    xt_d = nc.dram_tensor("xt", [D, S], F32, kind="ExternalInput").ap()
    wall_d = nc.dram_tensor("wall", [NDC, 6, 128, DH], F32, kind="ExternalInput").ap()
    wo_d = nc.dram_tensor("wo", [HPC, DH, D], F32, kind="ExternalInput").ap()
    bq_d = nc.dram_tensor("bq", [HPC, DH, 1], F32, kind="ExternalInput").ap()
    bk_d = nc.dram_tensor("bk", [HPC, DH, 1], F32, kind="ExternalInput").ap()
    bv_d = nc.dram_tensor("bv", [HPC, DH, 1], F32, kind="ExternalInput").ap()
    cos_d = nc.dram_tensor("cost", [DH, S], F32, kind="ExternalInput").ap()
    sin_d = nc.dram_tensor("sint", [DH, S], F32, kind="ExternalInput").ap()
    msk_d = nc.dram_tensor("maskneg", [128, 896], F32, kind="ExternalInput").ap()
    pt_d = nc.dram_tensor("pt", [128, 128], F32, kind="ExternalInput").ap()
    id_d = nc.dram_tensor("ident", [128, 128], F32, kind="ExternalInput").ap()
    oc_d = nc.dram_tensor("onescol", [128, 1], F32, kind="ExternalInput").ap()
    or_d = nc.dram_tensor("onesrow", [2, 128], F32, kind="ExternalInput").ap()
    out_d = nc.dram_tensor("outt", [D, S], BF16, kind="ExternalOutput").ap()

    with tile.TileContext(nc) as tc:
        with tc.tile_pool(name="persist", bufs=1) as pp:
            # persistent SBUF
            wo_sb = []
            bqs, bks, bvs = [], [], []
            qrot, krot, v_sb, znt = [], [], [], []
            for h in range(HPC):
                wo_sb.append(pp.tile([DH, D], F32R, tag=f"wo{h}", name=f"wo{h}"))
                for lst, dd, nm in ((bqs, bq_d, "bq"), (bks, bk_d, "bk"), (bvs, bv_d, "bv")):
                    bt = pp.tile([DH, 1], F32, tag=f"{nm}{h}")
                    nc.scalar.dma_start(bt[:], dd[h])
                    lst.append(bt)
                qrot.append(pp.tile([DH, S], F32R, tag=f"qrot{h}", name=f"qrot{h}"))
                krot.append(pp.tile([DH, S], F32R, tag=f"krot{h}", name=f"krot{h}"))
                v_sb.append(pp.tile([128, NKC, DH], F32R, tag=f"v{h}", name=f"v{h}"))
                znt.append(pp.tile([DH, S], F32R, tag=f"znt{h}", name=f"znt{h}"))
            msk_sb = pp.tile([128, 896], F32, tag="msk")
            pt_sb = pp.tile([128, 128], F32R, tag="pt")
            nc.scalar.dma_start(pt_sb[:], pt_d[:].bitcast(F32R))
            id_sb = pp.tile([128, 128], F32R, tag="ident")
            nc.scalar.dma_start(id_sb[:], id_d[:].bitcast(F32R))
            oc_sb = pp.tile([128, 1], F32R, tag="onescol")
            nc.scalar.dma_start(oc_sb[:], oc_d[:].bitcast(F32R))
            or_sb_full = pp.tile([2, 128], F32R, tag="onesrow")
            nc.scalar.dma_start(or_sb_full[:], or_d[:].bitcast(F32R))
            or_sb = or_sb_full[0:1, :]
            cabs_sb = pp.tile([128, 1], mybir.dt.uint32, tag="cabs")
            nc.vector.memset(cabs_sb[:], 0x7FFFFFFF)
            csgn_sb = pp.tile([128, 1], mybir.dt.uint32, tag="csgn")
            nc.vector.memset(csgn_sb[:], 0x80000000)
            cone_sb = pp.tile([128, 1], mybir.dt.uint32, tag="cone")
            nc.vector.memset(cone_sb[:], 0x3F800000)
            ln2_sb = pp.tile([128, 1], F32, tag="ln2")
            nc.vector.memset(ln2_sb[:], LN2)
            zero_sb = pp.tile([128, 1], F32, tag="zero")
            nc.vector.memset(zero_sb[:], 0.0)
            import os as _os
            _nonce = float(int(_os.environ.get("KBUILD_NONCE", "0")))
            nonce_sb = pp.tile([128, 1], F32, tag="nonce")
            nc.vector.memset(nonce_sb[:], _nonce)

            # ---------------- Phase A: projections + RoPE + V transpose ----
            with tc.tile_pool(name="aphase", bufs=1) as ap_, \
                 tc.tile_pool(name="axt", bufs=4) as axt, \
                 tc.tile_pool(name="aev", bufs=6) as aev, \
                 tc.tile_pool(name="arope", bufs=6) as arp, \
                 tc.tile_pool(name="psA", bufs=1, space="PSUM") as psA, \
                 tc.tile_pool(name="psShuf", bufs=1, space="PSUM") as psSh, \
                 tc.tile_pool(name="psVtr", bufs=1, space="PSUM") as psVt:
                wall_sb = ap_.tile([128, NDC, 6, DH], F32R, tag="wall", name="wall_sb")
                widx = {("q", 0): 0, ("q", 1): 1, ("k", 0): 2, ("k", 1): 3,
                        ("v", 0): 4, ("v", 1): 5}
                cos_sb = ap_.tile([DH, S], F32, tag="cos")
                sin_sb = ap_.tile([DH, S], F32, tag="sin")

                a_deferred = []
                for st in range(NST):
                    ssl = slice(st * ST, (st + 1) * ST)
                    acc = {}
                    for key in ("q", "k", "v"):
                        for h in range(HPC):
                            acc[(key, h)] = psA.tile([128, ST], F32, tag=f"acc{key}{h}", name=f"acc{key}{h}")
                    for dc in range(NDC):
                        if st == 0:
                            nc.gpsimd.dma_start(
                                wall_sb[:, dc, :, :],
                                wall_d[dc].rearrange("i p e -> p i e").bitcast(F32R))
                        xt_t = axt.tile([128, ST], F32R, tag="xt")
                        nc.sync.dma_start(
                            xt_t[:],
                            xt_d[dc * 128:(dc + 1) * 128, ssl].bitcast(F32R),
                        )
                        if st == 0 and dc == 4:
                            nc.scalar.dma_start(cos_sb[:], cos_d[:])
                            nc.scalar.dma_start(sin_sb[:], sin_d[:])
                        for key in ("q", "k", "v"):
                            for h in range(HPC):
                                nc.tensor.matmul(
                                    acc[(key, h)][:], wall_sb[:, dc, widx[(key, h)], :], xt_t[:],
                                    start=(dc == 0), stop=(dc == NDC - 1),
                                )
                        if a_deferred and dc % 2 == 1:
                            a_deferred.pop(0)()
                    # evictions free the acc slots now; the rope/transpose PE work is
                    # deferred into the next st's dc loop (A-phase software pipeline)
                    def make_rope(key, h, st, ssl, x_sb):
                        def run():
                            dst = (qrot if key == "q" else krot)[h]
                            shuf = psSh.tile([128, ST], F32, tag="shuf", name=f"sh{key}{h}_{st}")
                            nc.tensor.matmul(shuf[:], pt_sb[:], x_sb[:],
                                             start=True, stop=True)
                            t1 = arp.tile([128, ST], F32, tag="t1", name=f"t1{key}{h}_{st}")
                            nc.vector.tensor_tensor(t1[:], x_sb[:].bitcast(F32), cos_sb[:, ssl], ALU.mult)
                            t2 = arp.tile([128, ST], F32, tag="t2", name=f"t2{key}{h}_{st}")
                            nc.vector.tensor_tensor(t2[:], shuf[:], sin_sb[:, ssl], ALU.mult)
                            nc.vector.tensor_tensor(dst[:, ssl], t1[:], t2[:], ALU.add)
                        return run

                    def make_vtr(h, st, vt_sb):
                        def run():
                            for sc in range(ST // 128):
                                vtr = psVt.tile([128, 128], F32, tag="vtr", name=f"vtr{h}_{st}_{sc}")
                                nc.tensor.transpose(vtr[:].bitcast(F32R),
                                                    vt_sb[:, sc * 128:(sc + 1) * 128], id_sb[:])
                                nc.vector.tensor_copy(v_sb[h][:, st * 4 + sc, :], vtr[:])
                        return run

                    for key in ("q", "k"):
                        for h in range(HPC):
                            bias = (bqs if key == "q" else bks)[h]
                            x_sb = aev.tile([128, ST], F32R, tag="ev", bufs=8,
                                            name=f"ev{key}{h}_{st}")
                            nc.scalar.activation(x_sb[:], acc[(key, h)][:], AF.Identity, bias=bias[:])
                            a_deferred.append(make_rope(key, h, st, ssl, x_sb))
                    for h in range(HPC):
                        vt_sb = aev.tile([128, ST], F32R, tag="evv", bufs=4, name=f"evv{h}_{st}")
                        nc.vector.tensor_scalar(vt_sb[:], acc[("v", h)][:], bvs[h][:], None, ALU.add)
                        a_deferred.append(make_vtr(h, st, vt_sb))
                while a_deferred:
                    a_deferred.pop(0)()

            # deferred non-critical DMAs (needed in B/C only)
            nc.scalar.dma_start(msk_sb[:], msk_d[:])
            for h in range(HPC):
                nc.scalar.dma_start(wo_sb[h][:], wo_d[h].bitcast(F32R))

            # ---------------- Phases B + C --------------------------------
            # greedy engine balancing for per-block elementwise passes
            load = {"dve": 0.0, "act": 0.0, "gp": 0.0}

            def pick(*opts):
                e, c = min(opts, key=lambda ec: load[ec[0]] + ec[1])
                load[e] += c
                return e

            bpools = [tc.tile_pool(name="bwork", bufs=2),
                      tc.tile_pool(name="bsmall", bufs=2),
                      tc.tile_pool(name="psS", bufs=2, space="PSUM"),
                      tc.tile_pool(name="psZ", bufs=1, space="PSUM"),
                      tc.tile_pool(name="psD", bufs=1, space="PSUM"),
                      tc.tile_pool(name="psO", bufs=1, space="PSUM")]
            with tc.tile_pool(name="cout", bufs=3) as co:
                bw, bsm, psS, psZ, psD, psO = [p.__enter__() for p in bpools]
                U32 = mybir.dt.uint32
                LOOK = 2
                pss_map = {}
                psz = {}
                psd = {}

                def emit_scores(j, kc, h):
                    jj = kc - 4 * j
                    lo = jj * 128 if 0 <= jj < 4 else 0
                    pss = psS.tile([128, ST], F32, tag="s", name=f"s{h}_{j}_{kc}")
                    nc.tensor.matmul(pss[:, lo:], krot[h][:, kc * 128:(kc + 1) * 128],
                                     qrot[h][:, j * ST + lo:(j + 1) * ST],
                                     start=True, stop=True)
                    pss_map[(j, kc, h)] = (pss, lo)

                def emit_rest(j, kc, h):
                    pss, lo = pss_map.pop((j, kc, h))
                    lsl = slice(lo, ST)
                    wdt = ST - lo
                    f = wdt / ST
                    a = bw.tile([128, ST], F32, tag="a", bufs=4, name=f"a{h}_{j}_{kc}")
                    sg = bw.tile([128, ST], F32, tag="sg", bufs=4, name=f"sg{h}_{j}_{kc}")
                    if pick(("dve", 0.76 * f), ("act", 0.80 * f)) == "dve":
                        nc.vector.tensor_scalar(a[:, lsl].bitcast(U32), pss[:, lsl].bitcast(U32),
                                                cabs_sb[:], None, ALU.bitwise_and)
                        exp_scale = C_SCALE
                        load["act"] += 0.80 * f
                        nc.scalar.activation(sg[:, lsl], pss[:, lsl], AF.Sign, bias=zero_sb[:])
                    else:
                        nc.scalar.activation(a[:, lsl], pss[:, lsl], AF.Abs,
                                             bias=zero_sb[:], scale=C_SCALE)
                        exp_scale = 1.0
                        load["dve"] += 0.76 * f
                        nc.vector.tensor_scalar(sg[:, lsl].bitcast(U32), pss[:, lsl].bitcast(U32),
                                                csgn_sb[:], cone_sb[:],
                                                ALU.bitwise_and, ALU.bitwise_or)
                    jj = kc - 4 * j
                    if 0 <= jj < 4:
                        load["gp"] += 0.25
                        nc.gpsimd.tensor_tensor(
                            a[:, lo:lo + 128], a[:, lo:lo + 128],
                            msk_sb[:, 384:512], ALU.add)
                    e2 = bw.tile([128, ST], F32R, tag="e2", bufs=4, name=f"e2{h}_{j}_{kc}")
                    load["act"] += 0.80 * f
                    nc.scalar.activation(e2[:, lsl], a[:, lsl], AF.Exp, bias=ln2_sb[:],
                                         scale=exp_scale)
                    w = bw.tile([128, ST], F32R, tag="w", bufs=6, name=f"w{h}_{j}_{kc}")
                    if pick(("dve", 0.90 * f), ("gp", 0.95 * f)) == "dve":
                        nc.vector.tensor_tensor(w[:, lsl], sg[:, lsl],
                                                e2[:, lsl].bitcast(F32), ALU.mult)
                    else:
                        nc.gpsimd.tensor_tensor(w[:, lsl], sg[:, lsl],
                                                e2[:, lsl].bitcast(F32), ALU.mult)
                    zd_deferred.append((j, kc, h, lsl, e2, w))

                def emit_zd(j, kc, h, lsl, e2, w):
                    if kc == 0:
                        psz[(j, h)] = psZ.tile([128, ST], F32, tag=f"z{h}", name=f"z{h}_{j}")
                        psd[(j, h)] = psD.tile([1, ST], F32, tag=f"d{h}", name=f"d{h}_{j}")
                    nkc_ = 4 * (j + 1)
                    nc.tensor.matmul(psd[(j, h)][:, lsl], oc_sb[:], e2[:, lsl],
                                     start=(kc == 0), stop=(kc == nkc_ - 1))
                    nc.tensor.matmul(psz[(j, h)][:, lsl], v_sb[h][:, kc, :], w[:, lsl],
                                     start=(kc == 0), stop=(kc == nkc_ - 1))

                def finalize(j, h):
                    jsl = slice(j * ST, (j + 1) * ST)
                    r32 = bsm.tile([1, ST], F32, tag="r32", name=f"r32_{h}_{j}")
                    load["dve"] += 0.75
                    nc.vector.reciprocal_approx_fast(r32[:], psd.pop((j, h))[:])
                    rb_sb = bw.tile([128, ST], F32, tag="rb", name=f"rbs{h}_{j}")
                    load["gp"] += 0.95
                    nc.gpsimd.partition_broadcast(rb_sb[:], r32[:])
                    load["dve"] += 0.87
                    nc.vector.tensor_tensor(znt[h][:, jsl], psz.pop((j, h))[:], rb_sb[:], ALU.mult)

                c_deferred = []

                def emit_C(j, pool=None):
                    pool_ = pool
                    jsl = slice(j * ST, (j + 1) * ST)
                    for mc0 in range(0, D // 128, 2):
                        c_deferred.append(make_Cpair(j, jsl, mc0, pool_))

                def make_Cpair(j, jsl, mc0, pool_):
                    def run():
                        # two pso banks interleaved: avoids same-bank accumulate stalls
                        ps_pair = [(pool_ or psO).tile([128, ST], F32, tag="o", bufs=2,
                                    name=f"o{j}_{mc0 + k}") for k in range(2)]
                        for h in range(HPC):
                            for k in range(2):
                                mc = mc0 + k
                                nc.tensor.matmul(ps_pair[k][:],
                                                 wo_sb[h][:, mc * 128:(mc + 1) * 128],
                                                 znt[h][:, jsl], start=(h == 0),
                                                 stop=(h == HPC - 1))
                        for k in range(2):
                            mc = mc0 + k
                            o_sb = co.tile([128, ST], BF16, tag="o", name=f"ev{j}_{mc}")
                            if pick(("dve", 0.76), ("act", 0.80)) == "dve":
                                nc.vector.tensor_copy(o_sb[:], ps_pair[k][:])
                            else:
                                nc.scalar.activation(o_sb[:], ps_pair[k][:], AF.Copy)
                            nc.sync.dma_start(out_d[mc * 128:(mc + 1) * 128, jsl], o_sb[:])
                    return run

                blocks = [(j, kc, h) for j in range(NST)
                          for kc in range(4 * (j + 1)) for h in range(HPC)]
                zd_deferred = []
                ZLAG = 2
                done_in_j = {j: 0 for j in range(NST)}

                def pump_zd(limit=1):
                    while len(zd_deferred) > ZLAG if limit is None else \
                            (limit > 0 and len(zd_deferred) > ZLAG):
                        args = zd_deferred.pop(0)
                        emit_zd(*args)
                        after_rest(args[0])
                        if limit is not None:
                            limit -= 1

                fin_pending = []   # [j, ticks, stage]

                def after_rest(j):
                    done_in_j[j] += 1
                    if done_in_j[j] == 2 * 4 * (j + 1):
                        fin_pending.append([j, 0, 0])

                def fin_tick(force=False):
                    for e in fin_pending:
                        e[1] += 1
                    if not fin_pending:
                        return
                    j_, t_, stage = fin_pending[0]
                    if stage == 0 and (force or t_ >= 1):
                        finalize(j_, 0)
                        fin_pending[0][2] = 1
                    elif stage == 1 and (force or t_ >= 4):
                        fin_pending.pop(0)
                        finalize(j_, 1)
                        emit_C(j_)

                for i, (j, kc, h) in enumerate(blocks):
                    emit_scores(j, kc, h)
                    if i >= LOOK:
                        jj_, kc_, h_ = blocks[i - LOOK]
                        emit_rest(jj_, kc_, h_)
                        pump_zd(1)
                    fin_tick()
                    if c_deferred:
                        c_deferred.pop(0)()
                for (j, kc, h) in blocks[len(blocks) - LOOK:]:
                    emit_rest(j, kc, h)
                    pump_zd(1)
                while zd_deferred:
                    args = zd_deferred.pop(0)
                    emit_zd(*args)
                    after_rest(args[0])
                    fin_tick()
                while fin_pending:
                    fin_tick(force=True)
                    fin_tick(force=True)
                while c_deferred:
                    c_deferred.pop(0)()
                for p in reversed(bpools):
                    p.__exit__(None, None, None)
    nc.compile()
    return nc


def _host_constants():
    inv = 1.0 / (10000.0 ** (np.arange(0, DH, 2, dtype=np.float32) / DH))
    t = np.arange(S, dtype=np.float32)
    fr = t[:, None] * inv[None, :]                       # [S, DH/2]
    cosT = np.repeat(np.cos(fr).astype(np.float32).T, 2, axis=0)  # [DH, S]
    sinT = np.repeat(np.sin(fr).astype(np.float32).T, 2, axis=0)

    # sliding causal mask: msk[k, c] = 0 if k <= c - 384 else MASK_NEG
    kk = np.arange(128)[:, None]
    cc = np.arange(896)[None, :]
    msk = np.where(kk <= cc - 384, 0.0, MASK_NEG).astype(np.float32)

    # pt = P.T with P@x the rotate-half shuffle: (P x)[2i] = -x[2i+1], (P x)[2i+1] = x[2i]
    pt = np.zeros((128, 128), dtype=np.float32)
    i = np.arange(0, 128, 2)
    pt[i + 1, i] = -1.0
    pt[i, i + 1] = 1.0

    ident = np.eye(128, dtype=np.float32)
    onescol = np.ones((128, 1), dtype=np.float32)
    onesrow = np.ones((2, 128), dtype=np.float32)  # rb = 1/D2; row0 used; padded for cache key
    return cosT, sinT, msk, pt, ident, onescol, onesrow


def _run(inputs, trace=False, trace_kwargs=None):
    from concourse.bass_utils import run_bass_kernel_spmd

    if "nc" not in _CACHE:
        _CACHE["nc"] = _build_program()
    nc = _CACHE["nc"]

    resid_pre = np.asarray(inputs["resid_pre"], dtype=np.float32)
    W_Q = np.asarray(inputs["W_Q"], dtype=np.float32)
    W_K = np.asarray(inputs["W_K"], dtype=np.float32)
    W_V = np.asarray(inputs["W_V"], dtype=np.float32)
    W_O = np.asarray(inputs["W_O"], dtype=np.float32)
    b_Q = np.asarray(inputs["b_Q"], dtype=np.float32)
    b_K = np.asarray(inputs["b_K"], dtype=np.float32)
    b_V = np.asarray(inputs["b_V"], dtype=np.float32)
    b_O = np.asarray(inputs["b_O"], dtype=np.float32)

    xt = np.ascontiguousarray(resid_pre[0].T)
    cosT, sinT, msk, pt, ident, onescol, onesrow = _host_constants()

    in_maps = []
    for c in range(NC):
        hs = slice(c * HPC, (c + 1) * HPC)
        wl = np.empty((NDC, 6, 128, DH), dtype=np.float32)
        for dc in range(NDC):
            sl = slice(dc * 128, (dc + 1) * 128)
            wl[dc, 0] = W_Q[c * HPC + 0][sl]
            wl[dc, 1] = W_Q[c * HPC + 1][sl]
            wl[dc, 2] = W_K[c * HPC + 0][sl]
            wl[dc, 3] = W_K[c * HPC + 1][sl]
            wl[dc, 4] = W_V[c * HPC + 0][sl]
            wl[dc, 5] = W_V[c * HPC + 1][sl]
        in_maps.append({
            "xt": xt,
            "wall": wl,
            "wo": np.ascontiguousarray(W_O[hs]),
            "bq": np.ascontiguousarray(b_Q[hs][:, :, None]),
            "bk": np.ascontiguousarray(b_K[hs][:, :, None]),
            "bv": np.ascontiguousarray(b_V[hs][:, :, None]),
            "cost": cosT, "sint": sinT, "maskneg": msk, "pt": pt,
            "ident": ident, "onescol": onescol, "onesrow": onesrow,
        })

    kw = dict(trace_kwargs or {})
    last_err = None
    for attempt in range(3):
        try:
            res = run_bass_kernel_spmd(nc, in_maps, list(range(NC)), trace=trace, **kw)
            break
        except Exception as e:  # transient NRT_EXEC_UNIT_UNRECOVERABLE wedges clear on retry
            last_err = e
            if attempt == 2 or "UNRECOVERABLE" not in str(e).upper() and "UNAVAILABLE" not in str(e).upper():
                raise
            import time
            time.sleep(3.0)
    else:
        raise last_err

    acc = np.zeros((D, S), dtype=np.float32)
    for c in range(NC):
        acc += np.asarray(res.results[c]["outt"]).astype(np.float32)
    out = acc.T + b_O[None, :]
    return out.reshape(1, S, D).astype(np.float32), res


def kernel(**inputs) -> np.ndarray:
    out, _ = _run(inputs, trace=False)
    return out


# revision 37
# speedup vs baseline: 1.4563x; 1.4563x over previous
"""Trainium2 Bass kernel for nn_Attention_10015863734775.

Multi-head causal attention (16 heads, d_model 2048, d_head 128, seq 2048,
batch 1) with llama-style interleaved RoPE and a signed-softmax:
    attn_w = sign(s) * exp(|s| - max|s|);  attn = attn_w / (sum|attn_w| + 1e-6)
The max-subtraction cancels in the normalization (scores are O(5), exp is
safe in fp32), so the device computes attn = sign(s)exp(|s|) / sum exp(|s|).

Sharding: 2 heads per NeuronCore (8 cores). Each core receives the full
transposed residual X^T plus its head slices of W_Q/K/V/W_O and computes a
partial output projection outT_c[m, s] (bf16); the host sums the 8 partials
in f32, transposes, and adds b_O (exact: b_O enters after all nonlinearities).

Device layouts are all [feature, seq] ("T" layouts) so that:
  - scoresT[k, q] blocks come straight from matmul(lhsT=kT chunk, rhs=qT)
  - the z matmul needs no transposes (V is transposed once via the PE)
  - the signed-softmax k-sum is a ones-vector matmul on the PE
Causal masking skips fully-masked blocks; diagonal blocks add a -1e5 mask
to |s| before exp (exp underflows to exactly 0).
All matmuls run as float32r (~tf32 operand rounding, full fp32 accumulate).

Deltas over the original baseline (each measured):
  - reciprocal -> reciprocal_approx_fast (~5x) + f32r rounding copy
  - output staging + DMA in bf16 (halves output traffic; host sums in f32)
  - causal-mask adds on GPSIMD (SBUF-only op; frees DVE/ACT cycles)
"""

import math

import numpy as np

S = 2048          # sequence length
D = 2048          # d_model
DH = 128          # d_head
NH = 16           # total heads
NC = 8            # neuron cores
HPC = NH // NC    # heads per core (2)
ST = 512          # seq tile (matmul free dim / one PSUM bank)
NST = S // ST     # 4 seq tiles
NDC = D // 128    # 16 contraction chunks
NKC = S // 128    # 16 k chunks
C_SCALE = 1.0 / math.sqrt(float(DH))
LN2 = math.log(2.0)
MASK_NEG = -1.0e5

_CACHE = {}


def _build_program():
    import concourse.tile as tile
    from concourse import bacc, mybir

    F32 = mybir.dt.float32
    F32R = mybir.dt.float32r
    BF16 = mybir.dt.bfloat16
    AF = mybir.ActivationFunctionType
    ALU = mybir.AluOpType

    nc = bacc.Bacc("TRN2", target_bir_lowering=False, debug=False, num_devices=NC)

    xt_d = nc.dram_tensor("xt", [D, S], F32, kind="ExternalInput").ap()
    wall_d = nc.dram_tensor("wall", [NDC, 6, 128, DH], F32, kind="ExternalInput").ap()
    wo_d = nc.dram_tensor("wo", [HPC, DH, D], F32, kind="ExternalInput").ap()
    bq_d = nc.dram_tensor("bq", [HPC, DH, 1], F32, kind="ExternalInput").ap()
    bk_d = nc.dram_tensor("bk", [HPC, DH, 1], F32, kind="ExternalInput").ap()
    bv_d = nc.dram_tensor("bv", [HPC, DH, 1], F32, kind="ExternalInput").ap()
    cos_d = nc.dram_tensor("cost", [DH, S], F32, kind="ExternalInput").ap()
    sin_d = nc.dram_tensor("sint", [DH, S], F32, kind="ExternalInput").ap()
    msk_d = nc.dram_tensor("maskneg", [128, 896], F32, kind="ExternalInput").ap()
    pt_d = nc.dram_tensor("pt", [128, 128], F32, kind="ExternalInput").ap()
    id_d = nc.dram_tensor("ident", [128, 128], F32, kind="ExternalInput").ap()
    oc_d = nc.dram_tensor("onescol", [128, 1], F32, kind="ExternalInput").ap()
    or_d = nc.dram_tensor("onesrow", [2, 128], F32, kind="ExternalInput").ap()
    out_d = nc.dram_tensor("outt", [D, S], BF16, kind="ExternalOutput").ap()

    with tile.TileContext(nc) as tc:
        with tc.tile_pool(name="persist", bufs=1) as pp:
            # persistent SBUF
            wo_sb = []
            bqs, bks, bvs = [], [], []
            qrot, krot, v_sb, znt = [], [], [], []
            for h in range(HPC):
                wo_sb.append(pp.tile([DH, D], F32R, tag=f"wo{h}", name=f"wo{h}"))
                for lst, dd, nm in ((bqs, bq_d, "bq"), (bks, bk_d, "bk"), (bvs, bv_d, "bv")):
                    bt = pp.tile([DH, 1], F32, tag=f"{nm}{h}")
                    nc.scalar.dma_start(bt[:], dd[h])
                    lst.append(bt)
                qrot.append(pp.tile([DH, S], F32R, tag=f"qrot{h}", name=f"qrot{h}"))
                krot.append(pp.tile([DH, S], F32R, tag=f"krot{h}", name=f"krot{h}"))
                v_sb.append(pp.tile([128, NKC, DH], F32R, tag=f"v{h}", name=f"v{h}"))
                znt.append(pp.tile([DH, S], F32R, tag=f"znt{h}", name=f"znt{h}"))
            msk_sb = pp.tile([128, 896], F32, tag="msk")
            pt_sb = pp.tile([128, 128], F32R, tag="pt")
            nc.scalar.dma_start(pt_sb[:], pt_d[:].bitcast(F32R))
            id_sb = pp.tile([128, 128], F32R, tag="ident")
            nc.scalar.dma_start(id_sb[:], id_d[:].bitcast(F32R))
            oc_sb = pp.tile([128, 1], F32R, tag="onescol")
            nc.scalar.dma_start(oc_sb[:], oc_d[:].bitcast(F32R))
            or_sb_full = pp.tile([2, 128], F32R, tag="onesrow")
            nc.scalar.dma_start(or_sb_full[:], or_d[:].bitcast(F32R))
            or_sb = or_sb_full[0:1, :]
            cabs_sb = pp.tile([128, 1], mybir.dt.uint32, tag="cabs")
            nc.vector.memset(cabs_sb[:], 0x7FFFFFFF)
            csgn_sb = pp.tile([128, 1], mybir.dt.uint32, tag="csgn")
            nc.vector.memset(csgn_sb[:], 0x80000000)
            cone_sb = pp.tile([128, 1], mybir.dt.uint32, tag="cone")
            nc.vector.memset(cone_sb[:], 0x3F800000)
            ln2_sb = pp.tile([128, 1], F32, tag="ln2")
            nc.vector.memset(ln2_sb[:], LN2)
            zero_sb = pp.tile([128, 1], F32, tag="zero")
            nc.vector.memset(zero_sb[:], 0.0)
            import os as _os
            _nonce = float(int(_os.environ.get("KBUILD_NONCE", "0")))
            nonce_sb = pp.tile([128, 1], F32, tag="nonce")
            nc.vector.memset(nonce_sb[:], _nonce)

            # ---------------- Phase A: projections + RoPE + V transpose ----
            with tc.tile_pool(name="aphase", bufs=1) as ap_, \
                 tc.tile_pool(name="axt", bufs=4) as axt, \
                 tc.tile_pool(name="aev", bufs=6) as aev, \
                 tc.tile_pool(name="arope", bufs=6) as arp, \
                 tc.tile_pool(name="psA", bufs=1, space="PSUM") as psA, \
                 tc.tile_pool(name="psShuf", bufs=1, space="PSUM") as psSh, \
                 tc.tile_pool(name="psVtr", bufs=1, space="PSUM") as psVt:
                wall_sb = ap_.tile([128, NDC, 6, DH], F32R, tag="wall", name="wall_sb")
                widx = {("q", 0): 0, ("q", 1): 1, ("k", 0): 2, ("k", 1): 3,
                        ("v", 0): 4, ("v", 1): 5}
                cos_sb = ap_.tile([DH, S], F32, tag="cos")
                sin_sb = ap_.tile([DH, S], F32, tag="sin")

                a_deferred = []
                for st in range(NST):
                    ssl = slice(st * ST, (st + 1) * ST)
                    acc = {}
                    for key in ("q", "k", "v"):
                        for h in range(HPC):
                            acc[(key, h)] = psA.tile([128, ST], F32, tag=f"acc{key}{h}", name=f"acc{key}{h}")
                    for dc in range(NDC):
                        if st == 0:
                            nc.gpsimd.dma_start(
                                wall_sb[:, dc, :, :],
                                wall_d[dc].rearrange("i p e -> p i e").bitcast(F32R))
                        xt_t = axt.tile([128, ST], F32R, tag="xt")
                        nc.sync.dma_start(
                            xt_t[:],
                            xt_d[dc * 128:(dc + 1) * 128, ssl].bitcast(F32R),
                        )
                        if st == 0 and dc == 4:
                            nc.scalar.dma_start(cos_sb[:], cos_d[:])
                            nc.scalar.dma_start(sin_sb[:], sin_d[:])
                        for key in ("q", "k", "v"):
                            for h in range(HPC):
                                nc.tensor.matmul(
                                    acc[(key, h)][:], wall_sb[:, dc, widx[(key, h)], :], xt_t[:],
                                    start=(dc == 0), stop=(dc == NDC - 1),
                                )
                        if a_deferred and dc % 2 == 1:
                            a_deferred.pop(0)()
                    # evictions free the acc slots now; the rope/transpose PE work is
                    # deferred into the next st's dc loop (A-phase software pipeline)
                    def make_rope(key, h, st, ssl, x_sb):
                        def run():
                            dst = (qrot if key == "q" else krot)[h]
                            shuf = psSh.tile([128, ST], F32, tag="shuf", name=f"sh{key}{h}_{st}")
                            nc.tensor.matmul(shuf[:], pt_sb[:], x_sb[:],
                                             start=True, stop=True)
                            t1 = arp.tile([128, ST], F32, tag="t1", name=f"t1{key}{h}_{st}")
                            nc.vector.tensor_tensor(t1[:], x_sb[:].bitcast(F32), cos_sb[:, ssl], ALU.mult)
                            t2 = arp.tile([128, ST], F32, tag="t2", name=f"t2{key}{h}_{st}")
                            nc.vector.tensor_tensor(t2[:], shuf[:], sin_sb[:, ssl], ALU.mult)
                            nc.vector.tensor_tensor(dst[:, ssl], t1[:], t2[:], ALU.add)
                        return run

                    def make_vtr(h, st, vt_sb):
                        def run():
                            for sc in range(ST // 128):
                                vtr = psVt.tile([128, 128], F32, tag="vtr", name=f"vtr{h}_{st}_{sc}")
                                nc.tensor.transpose(vtr[:].bitcast(F32R),
                                                    vt_sb[:, sc * 128:(sc + 1) * 128], id_sb[:])
                                nc.vector.tensor_copy(v_sb[h][:, st * 4 + sc, :], vtr[:])
                        return run

                    for key in ("q", "k"):
                        for h in range(HPC):
                            bias = (bqs if key == "q" else bks)[h]
                            x_sb = aev.tile([128, ST], F32R, tag="ev", bufs=8,
                                            name=f"ev{key}{h}_{st}")
                            nc.scalar.activation(x_sb[:], acc[(key, h)][:], AF.Identity, bias=bias[:])
                            a_deferred.append(make_rope(key, h, st, ssl, x_sb))
                    for h in range(HPC):
                        vt_sb = aev.tile([128, ST], F32R, tag="evv", bufs=4, name=f"evv{h}_{st}")
                        nc.vector.tensor_scalar(vt_sb[:], acc[("v", h)][:], bvs[h][:], None, ALU.add)
                        a_deferred.append(make_vtr(h, st, vt_sb))
                while a_deferred:
                    a_deferred.pop(0)()

            # deferred non-critical DMAs (needed in B/C only)
            nc.scalar.dma_start(msk_sb[:], msk_d[:])
            for h in range(HPC):
                nc.scalar.dma_start(wo_sb[h][:], wo_d[h].bitcast(F32R))

            # ---------------- Phases B + C --------------------------------
            # greedy engine balancing for per-block elementwise passes
            load = {"dve": 0.0, "act": 0.0, "gp": 0.0}

            def pick(*opts):
                e, c = min(opts, key=lambda ec: load[ec[0]] + ec[1])
                load[e] += c
                return e

            bpools = [tc.tile_pool(name="bwork", bufs=2),
                      tc.tile_pool(name="bsmall", bufs=2),
                      tc.tile_pool(name="psS", bufs=2, space="PSUM"),
                      tc.tile_pool(name="psZ", bufs=1, space="PSUM"),
                      tc.tile_pool(name="psD", bufs=1, space="PSUM"),
                      tc.tile_pool(name="psO", bufs=1, space="PSUM")]
            with tc.tile_pool(name="cout", bufs=3) as co:
                bw, bsm, psS, psZ, psD, psO = [p.__enter__() for p in bpools]
                U32 = mybir.dt.uint32
                LOOK = 2
                pss_map = {}
                psz = {}
                psd = {}

                def emit_scores(j, kc, h):
                    jj = kc - 4 * j
                    lo = jj * 128 if 0 <= jj < 4 else 0
                    pss = psS.tile([128, ST], F32, tag="s", name=f"s{h}_{j}_{kc}")
                    nc.tensor.matmul(pss[:, lo:], krot[h][:, kc * 128:(kc + 1) * 128],
                                     qrot[h][:, j * ST + lo:(j + 1) * ST],
                                     start=True, stop=True)
                    pss_map[(j, kc, h)] = (pss, lo)

                def emit_rest(j, kc, h):
                    pss, lo = pss_map.pop((j, kc, h))
                    lsl = slice(lo, ST)
                    wdt = ST - lo
                    f = wdt / ST
                    a = bw.tile([128, ST], F32, tag="a", bufs=4, name=f"a{h}_{j}_{kc}")
                    sg = bw.tile([128, ST], F32, tag="sg", bufs=4, name=f"sg{h}_{j}_{kc}")
                    if pick(("dve", 0.76 * f), ("act", 0.80 * f)) == "dve":
                        nc.vector.tensor_scalar(a[:, lsl].bitcast(U32), pss[:, lsl].bitcast(U32),
                                                cabs_sb[:], None, ALU.bitwise_and)
                        exp_scale = C_SCALE
                        load["act"] += 0.80 * f
                        nc.scalar.activation(sg[:, lsl], pss[:, lsl], AF.Sign, bias=zero_sb[:])
                    else:
                        nc.scalar.activation(a[:, lsl], pss[:, lsl], AF.Abs,
                                             bias=zero_sb[:], scale=C_SCALE)
                        exp_scale = 1.0
                        load["dve"] += 0.76 * f
                        nc.vector.tensor_scalar(sg[:, lsl].bitcast(U32), pss[:, lsl].bitcast(U32),
                                                csgn_sb[:], cone_sb[:],
                                                ALU.bitwise_and, ALU.bitwise_or)
                    jj = kc - 4 * j
                    if 0 <= jj < 4:
                        load["gp"] += 0.25
                        nc.gpsimd.tensor_tensor(
                            a[:, lo:lo + 128], a[:, lo:lo + 128],
                            msk_sb[:, 384:512], ALU.add)
                    e2 = bw.tile([128, ST], F32R, tag="e2", bufs=4, name=f"e2{h}_{j}_{kc}")
                    load["act"] += 0.80 * f
                    nc.scalar.activation(e2[:, lsl], a[:, lsl], AF.Exp, bias=ln2_sb[:],
                                         scale=exp_scale)
                    w = bw.tile([128, ST], F32R, tag="w", bufs=6, name=f"w{h}_{j}_{kc}")
                    if pick(("dve", 0.90 * f), ("gp", 0.95 * f)) == "dve":
                        nc.vector.tensor_tensor(w[:, lsl], sg[:, lsl],
                                                e2[:, lsl].bitcast(F32), ALU.mult)
                    else:
                        nc.gpsimd.tensor_tensor(w[:, lsl], sg[:, lsl],
                                                e2[:, lsl].bitcast(F32), ALU.mult)
                    zd_deferred.append((j, kc, h, lsl, e2, w))

                def emit_zd(j, kc, h, lsl, e2, w):
                    if kc == 0:
                        psz[(j, h)] = psZ.tile([128, ST], F32, tag=f"z{h}", name=f"z{h}_{j}")
                        psd[(j, h)] = psD.tile([1, ST], F32, tag=f"d{h}", name=f"d{h}_{j}")
                    nkc_ = 4 * (j + 1)
                    nc.tensor.matmul(psd[(j, h)][:, lsl], oc_sb[:], e2[:, lsl],
                                     start=(kc == 0), stop=(kc == nkc_ - 1))
                    nc.tensor.matmul(psz[(j, h)][:, lsl], v_sb[h][:, kc, :], w[:, lsl],
                                     start=(kc == 0), stop=(kc == nkc_ - 1))

                def finalize(j, h):
                    jsl = slice(j * ST, (j + 1) * ST)
                    r32 = bsm.tile([1, ST], F32, tag="r32", name=f"r32_{h}_{j}")
                    load["dve"] += 0.75
                    nc.vector.reciprocal_approx_fast(r32[:], psd.pop((j, h))[:])
                    r_sb = bsm.tile([1, ST], F32R, tag="rsb", name=f"r{h}_{j}")
                    nc.vector.tensor_copy(r_sb[:], r32[:])
                    psrb = psO.tile([128, ST], F32, tag="o", bufs=2, name=f"rb{h}_{j}")
                    nc.tensor.matmul(psrb[:], or_sb, r_sb[:], start=True, stop=True)
                    rb_sb = bw.tile([128, ST], F32, tag="rb", name=f"rbs{h}_{j}")
                    if pick(("dve", 0.76), ("act", 0.80)) == "dve":
                        nc.vector.tensor_copy(rb_sb[:], psrb[:])
                    else:
                        nc.scalar.activation(rb_sb[:], psrb[:], AF.Copy)
                    load["dve"] += 0.87
                    nc.vector.tensor_tensor(znt[h][:, jsl], psz.pop((j, h))[:], rb_sb[:], ALU.mult)

                c_deferred = []

                def emit_C(j, pool=None):
                    pool_ = pool
                    jsl = slice(j * ST, (j + 1) * ST)
                    for mc0 in range(0, D // 128, 2):
                        c_deferred.append(make_Cpair(j, jsl, mc0, pool_))

                def make_Cpair(j, jsl, mc0, pool_):
                    def run():
                        # two pso banks interleaved: avoids same-bank accumulate stalls
                        ps_pair = [(pool_ or psO).tile([128, ST], F32, tag="o", bufs=2,
                                    name=f"o{j}_{mc0 + k}") for k in range(2)]
                        for h in range(HPC):
                            for k in range(2):
                                mc = mc0 + k
                                nc.tensor.matmul(ps_pair[k][:],
                                                 wo_sb[h][:, mc * 128:(mc + 1) * 128],
                                                 znt[h][:, jsl], start=(h == 0),
                                                 stop=(h == HPC - 1))
                        for k in range(2):
                            mc = mc0 + k
                            o_sb = co.tile([128, ST], BF16, tag="o", name=f"ev{j}_{mc}")
                            if pick(("dve", 0.76), ("act", 0.80)) == "dve":
                                nc.vector.tensor_copy(o_sb[:], ps_pair[k][:])
                            else:
                                nc.scalar.activation(o_sb[:], ps_pair[k][:], AF.Copy)
                            nc.sync.dma_start(out_d[mc * 128:(mc + 1) * 128, jsl], o_sb[:])
                    return run

                blocks = [(j, kc, h) for j in range(NST)
                          for kc in range(4 * (j + 1)) for h in range(HPC)]
                zd_deferred = []
                ZLAG = 2
                done_in_j = {j: 0 for j in range(NST)}

                def pump_zd(limit=1):
                    while len(zd_deferred) > ZLAG if limit is None else \
                            (limit > 0 and len(zd_deferred) > ZLAG):
                        args = zd_deferred.pop(0)
                        emit_zd(*args)
                        after_rest(args[0])
                        if limit is not None:
                            limit -= 1

                fin_pending = []   # [j, ticks, stage]

                def after_rest(j):
                    done_in_j[j] += 1
                    if done_in_j[j] == 2 * 4 * (j + 1):
                        fin_pending.append([j, 0, 0])

                def fin_tick(force=False):
                    for e in fin_pending:
                        e[1] += 1
                    if not fin_pending:
                        return
                    j_, t_, stage = fin_pending[0]
                    if stage == 0 and (force or t_ >= 1):
                        finalize(j_, 0)
                        fin_pending[0][2] = 1
                    elif stage == 1 and (force or t_ >= 4):
                        fin_pending.pop(0)
                        finalize(j_, 1)
                        emit_C(j_)

                for i, (j, kc, h) in enumerate(blocks):
                    emit_scores(j, kc, h)
                    if i >= LOOK:
                        jj_, kc_, h_ = blocks[i - LOOK]
                        emit_rest(jj_, kc_, h_)
                        pump_zd(1)
                    fin_tick()
                    if c_deferred:
                        c_deferred.pop(0)()
                for (j, kc, h) in blocks[len(blocks) - LOOK:]:
                    emit_rest(j, kc, h)
                    pump_zd(1)
                while zd_deferred:
                    args = zd_deferred.pop(0)
                    emit_zd(*args)
                    after_rest(args[0])
                    fin_tick()
                while fin_pending:
                    fin_tick(force=True)
                    fin_tick(force=True)
                while c_deferred:
                    c_deferred.pop(0)()
                for p in reversed(bpools):
                    p.__exit__(None, None, None)
    nc.compile()
    return nc


def _host_constants():
    inv = 1.0 / (10000.0 ** (np.arange(0, DH, 2, dtype=np.float32) / DH))
    t = np.arange(S, dtype=np.float32)
    fr = t[:, None] * inv[None, :]                       # [S, DH/2]
    cosT = np.repeat(np.cos(fr).astype(np.float32).T, 2, axis=0)  # [DH, S]
    sinT = np.repeat(np.sin(fr).astype(np.float32).T, 2, axis=0)

    # sliding causal mask: msk[k, c] = 0 if k <= c - 384 else MASK_NEG
    kk = np.arange(128)[:, None]
    cc = np.arange(896)[None, :]
    msk = np.where(kk <= cc - 384, 0.0, MASK_NEG).astype(np.float32)

    # pt = P.T with P@x the rotate-half shuffle: (P x)[2i] = -x[2i+1], (P x)[2i+1] = x[2i]
    pt = np.zeros((128, 128), dtype=np.float32)
    i = np.arange(0, 128, 2)
    pt[i + 1, i] = -1.0
    pt[i, i + 1] = 1.0

    ident = np.eye(128, dtype=np.float32)
    onescol = np.ones((128, 1), dtype=np.float32)
    onesrow = np.ones((2, 128), dtype=np.float32)  # rb = 1/D2; row0 used; padded for cache key
    return cosT, sinT, msk, pt, ident, onescol, onesrow


def _run(inputs, trace=False, trace_kwargs=None):
    from concourse.bass_utils import run_bass_kernel_spmd

    if "nc" not in _CACHE:
        _CACHE["nc"] = _build_program()
    nc = _CACHE["nc"]

    resid_pre = np.asarray(inputs["resid_pre"], dtype=np.float32)
    W_Q = np.asarray(inputs["W_Q"], dtype=np.float32)
    W_K = np.asarray(inputs["W_K"], dtype=np.float32)
    W_V = np.asarray(inputs["W_V"], dtype=np.float32)
    W_O = np.asarray(inputs["W_O"], dtype=np.float32)
    b_Q = np.asarray(inputs["b_Q"], dtype=np.float32)
    b_K = np.asarray(inputs["b_K"], dtype=np.float32)
    b_V = np.asarray(inputs["b_V"], dtype=np.float32)
    b_O = np.asarray(inputs["b_O"], dtype=np.float32)

    xt = np.ascontiguousarray(resid_pre[0].T)
    cosT, sinT, msk, pt, ident, onescol, onesrow = _host_constants()

    in_maps = []
    for c in range(NC):
        hs = slice(c * HPC, (c + 1) * HPC)
        wl = np.empty((NDC, 6, 128, DH), dtype=np.float32)
        for dc in range(NDC):
            sl = slice(dc * 128, (dc + 1) * 128)
            wl[dc, 0] = W_Q[c * HPC + 0][sl]
            wl[dc, 1] = W_Q[c * HPC + 1][sl]
            wl[dc, 2] = W_K[c * HPC + 0][sl]
            wl[dc, 3] = W_K[c * HPC + 1][sl]
            wl[dc, 4] = W_V[c * HPC + 0][sl]
            wl[dc, 5] = W_V[c * HPC + 1][sl]
        in_maps.append({
            "xt": xt,
            "wall": wl,
            "wo": np.ascontiguousarray(W_O[hs]),
            "bq": np.ascontiguousarray(b_Q[hs][:, :, None]),
            "bk": np.ascontiguousarray(b_K[hs][:, :, None]),
            "bv": np.ascontiguousarray(b_V[hs][:, :, None]),
            "cost": cosT, "sint": sinT, "maskneg": msk, "pt": pt,
            "ident": ident, "onescol": onescol, "onesrow": onesrow,
        })

    kw = dict(trace_kwargs or {})
    last_err = None
    for attempt in range(3):
        try:
            res = run_bass_kernel_spmd(nc, in_maps, list(range(NC)), trace=trace, **kw)
            break
        except Exception as e:  # transient NRT_EXEC_UNIT_UNRECOVERABLE wedges clear on retry
            last_err = e
            if attempt == 2 or "UNRECOVERABLE" not in str(e).upper() and "UNAVAILABLE" not in str(e).upper():
                raise
            import time
            time.sleep(3.0)
    else:
        raise last_err

    acc = np.zeros((D, S), dtype=np.float32)
    for c in range(NC):
        acc += np.asarray(res.results[c]["outt"]).astype(np.float32)
    out = acc.T + b_O[None, :]
    return out.reshape(1, S, D).astype(np.float32), res


def kernel(**inputs) -> np.ndarray:
    out, _ = _run(inputs, trace=False)
    return out


# revision 38
# speedup vs baseline: 1.4847x; 1.0195x over previous
"""Trainium2 Bass kernel for nn_Attention_10015863734775.

Multi-head causal attention (16 heads, d_model 2048, d_head 128, seq 2048,
batch 1) with llama-style interleaved RoPE and a signed-softmax:
    attn_w = sign(s) * exp(|s| - max|s|);  attn = attn_w / (sum|attn_w| + 1e-6)
The max-subtraction cancels in the normalization (scores are O(5), exp is
safe in fp32), so the device computes attn = sign(s)exp(|s|) / sum exp(|s|).

Sharding: 2 heads per NeuronCore (8 cores). Each core receives the full
transposed residual X^T plus its head slices of W_Q/K/V/W_O and computes a
partial output projection outT_c[m, s] (bf16); the host sums the 8 partials
in f32, transposes, and adds b_O (exact: b_O enters after all nonlinearities).

Device layouts are all [feature, seq] ("T" layouts) so that:
  - scoresT[k, q] blocks come straight from matmul(lhsT=kT chunk, rhs=qT)
  - the z matmul needs no transposes (V is transposed once via the PE)
  - the signed-softmax k-sum is a ones-vector matmul on the PE
Causal masking skips fully-masked blocks; diagonal blocks add a -1e5 mask
to |s| before exp (exp underflows to exactly 0).
All matmuls run as float32r (~tf32 operand rounding, full fp32 accumulate).

Deltas over the original baseline (each measured):
  - reciprocal -> reciprocal_approx_fast (~5x) + f32r rounding copy
  - output staging + DMA in bf16 (halves output traffic; host sums in f32)
  - causal-mask adds on GPSIMD (SBUF-only op; frees DVE/ACT cycles)
"""

import math

import numpy as np

S = 2048          # sequence length
D = 2048          # d_model
DH = 128          # d_head
NH = 16           # total heads
NC = 8            # neuron cores
HPC = NH // NC    # heads per core (2)
ST = 512          # seq tile (matmul free dim / one PSUM bank)
NST = S // ST     # 4 seq tiles
NDC = D // 128    # 16 contraction chunks
NKC = S // 128    # 16 k chunks
C_SCALE = 1.0 / math.sqrt(float(DH))
LN2 = math.log(2.0)
MASK_NEG = -1.0e5

_CACHE = {}


def _build_program():
    import concourse.tile as tile
    from concourse import bacc, mybir

    F32 = mybir.dt.float32
    F32R = mybir.dt.float32r
    BF16 = mybir.dt.bfloat16
    AF = mybir.ActivationFunctionType
    ALU = mybir.AluOpType

    nc = bacc.Bacc("TRN2", target_bir_lowering=False, debug=False, num_devices=NC)

    xt_d = nc.dram_tensor("xt", [D, S], F32, kind="ExternalInput").ap()
    wall_d = nc.dram_tensor("wall", [NDC, 6, 128, DH], F32, kind="ExternalInput").ap()
    wo_d = nc.dram_tensor("wo", [HPC, DH, D], F32, kind="ExternalInput").ap()
    bq_d = nc.dram_tensor("bq", [HPC, DH, 1], F32, kind="ExternalInput").ap()
    bk_d = nc.dram_tensor("bk", [HPC, DH, 1], F32, kind="ExternalInput").ap()
    bv_d = nc.dram_tensor("bv", [HPC, DH, 1], F32, kind="ExternalInput").ap()
    cos_d = nc.dram_tensor("cost", [DH, S], F32, kind="ExternalInput").ap()
    sin_d = nc.dram_tensor("sint", [DH, S], F32, kind="ExternalInput").ap()
    msk_d = nc.dram_tensor("maskneg", [128, 896], F32, kind="ExternalInput").ap()
    pt_d = nc.dram_tensor("pt", [128, 128], F32, kind="ExternalInput").ap()
    id_d = nc.dram_tensor("ident", [128, 128], F32, kind="ExternalInput").ap()
    oc_d = nc.dram_tensor("onescol", [128, 1], F32, kind="ExternalInput").ap()
    or_d = nc.dram_tensor("onesrow", [2, 128], F32, kind="ExternalInput").ap()
    out_d = nc.dram_tensor("outt", [D, S], BF16, kind="ExternalOutput").ap()

    with tile.TileContext(nc) as tc:
        with tc.tile_pool(name="persist", bufs=1) as pp:
            # persistent SBUF
            wo_sb = []
            bqs, bks, bvs = [], [], []
            qrot, krot, v_sb, znt = [], [], [], []
            for h in range(HPC):
                wo_sb.append(pp.tile([DH, D], F32R, tag=f"wo{h}", name=f"wo{h}"))
                for lst, dd, nm in ((bqs, bq_d, "bq"), (bks, bk_d, "bk"), (bvs, bv_d, "bv")):
                    bt = pp.tile([DH, 1], F32, tag=f"{nm}{h}")
                    nc.scalar.dma_start(bt[:], dd[h])
                    lst.append(bt)
                qrot.append(pp.tile([DH, S], F32R, tag=f"qrot{h}", name=f"qrot{h}"))
                krot.append(pp.tile([DH, S], F32R, tag=f"krot{h}", name=f"krot{h}"))
                v_sb.append(pp.tile([128, NKC, DH], F32R, tag=f"v{h}", name=f"v{h}"))
                znt.append(pp.tile([DH, S], F32R, tag=f"znt{h}", name=f"znt{h}"))
            msk_sb = pp.tile([128, 896], F32, tag="msk")
            pt_sb = pp.tile([128, 128], F32R, tag="pt")
            nc.scalar.dma_start(pt_sb[:], pt_d[:].bitcast(F32R))
            id_sb = pp.tile([128, 128], F32R, tag="ident")
            nc.scalar.dma_start(id_sb[:], id_d[:].bitcast(F32R))
            oc_sb = pp.tile([128, 1], F32R, tag="onescol")
            nc.scalar.dma_start(oc_sb[:], oc_d[:].bitcast(F32R))
            or_sb_full = pp.tile([2, 128], F32R, tag="onesrow")
            nc.scalar.dma_start(or_sb_full[:], or_d[:].bitcast(F32R))
            or_sb = or_sb_full[0:1, :]
            cabs_sb = pp.tile([128, 1], mybir.dt.uint32, tag="cabs")
            nc.vector.memset(cabs_sb[:], 0x7FFFFFFF)
            csgn_sb = pp.tile([128, 1], mybir.dt.uint32, tag="csgn")
            nc.vector.memset(csgn_sb[:], 0x80000000)
            cone_sb = pp.tile([128, 1], mybir.dt.uint32, tag="cone")
            nc.vector.memset(cone_sb[:], 0x3F800000)
            ln2_sb = pp.tile([128, 1], F32, tag="ln2")
            nc.vector.memset(ln2_sb[:], LN2)
            zero_sb = pp.tile([128, 1], F32, tag="zero")
            nc.vector.memset(zero_sb[:], 0.0)
            import os as _os
            _nonce = float(int(_os.environ.get("KBUILD_NONCE", "0")))
            nonce_sb = pp.tile([128, 1], F32, tag="nonce")
            nc.vector.memset(nonce_sb[:], _nonce)

            # ---------------- Phase A: projections + RoPE + V transpose ----
            with tc.tile_pool(name="aphase", bufs=1) as ap_, \
                 tc.tile_pool(name="axt", bufs=6) as axt, \
                 tc.tile_pool(name="aev", bufs=6) as aev, \
                 tc.tile_pool(name="arope", bufs=4) as arp, \
                 tc.tile_pool(name="psA", bufs=1, space="PSUM") as psA, \
                 tc.tile_pool(name="psShuf", bufs=1, space="PSUM") as psSh, \
                 tc.tile_pool(name="psVtr", bufs=1, space="PSUM") as psVt:
                wall_sb = ap_.tile([128, NDC, 6, DH], F32R, tag="wall", name="wall_sb")
                widx = {("q", 0): 0, ("q", 1): 1, ("k", 0): 2, ("k", 1): 3,
                        ("v", 0): 4, ("v", 1): 5}
                cos_sb = ap_.tile([DH, S], F32, tag="cos")
                sin_sb = ap_.tile([DH, S], F32, tag="sin")

                a_deferred = []
                for st in range(NST):
                    ssl = slice(st * ST, (st + 1) * ST)
                    acc = {}
                    for key in ("q", "k", "v"):
                        for h in range(HPC):
                            acc[(key, h)] = psA.tile([128, ST], F32, tag=f"acc{key}{h}", name=f"acc{key}{h}")
                    for dc in range(NDC):
                        if st == 0:
                            nc.gpsimd.dma_start(
                                wall_sb[:, dc, :, :],
                                wall_d[dc].rearrange("i p e -> p i e").bitcast(F32R))
                        xt_t = axt.tile([128, ST], F32R, tag="xt")
                        nc.sync.dma_start(
                            xt_t[:],
                            xt_d[dc * 128:(dc + 1) * 128, ssl].bitcast(F32R),
                        )
                        if st == 0 and dc == 4:
                            nc.scalar.dma_start(cos_sb[:], cos_d[:])
                            nc.scalar.dma_start(sin_sb[:], sin_d[:])
                        for key in ("q", "k", "v"):
                            for h in range(HPC):
                                nc.tensor.matmul(
                                    acc[(key, h)][:], wall_sb[:, dc, widx[(key, h)], :], xt_t[:],
                                    start=(dc == 0), stop=(dc == NDC - 1),
                                )
                        if a_deferred and dc % 2 == 1:
                            a_deferred.pop(0)()
                    # evictions free the acc slots now; the rope/transpose PE work is
                    # deferred into the next st's dc loop (A-phase software pipeline)
                    def make_rope(key, h, st, ssl, x_sb):
                        def run():
                            dst = (qrot if key == "q" else krot)[h]
                            shuf = psSh.tile([128, ST], F32, tag="shuf", name=f"sh{key}{h}_{st}")
                            nc.tensor.matmul(shuf[:], pt_sb[:], x_sb[:],
                                             start=True, stop=True)
                            t1 = arp.tile([128, ST], F32, tag="t1", name=f"t1{key}{h}_{st}")
                            nc.vector.tensor_tensor(t1[:], x_sb[:].bitcast(F32), cos_sb[:, ssl], ALU.mult)
                            t2 = arp.tile([128, ST], F32, tag="t2", name=f"t2{key}{h}_{st}")
                            nc.vector.tensor_tensor(t2[:], shuf[:], sin_sb[:, ssl], ALU.mult)
                            nc.vector.tensor_tensor(dst[:, ssl], t1[:], t2[:], ALU.add)
                        return run

                    def make_vtr(h, st, vt_sb):
                        def run():
                            for sc in range(ST // 128):
                                vtr = psVt.tile([128, 128], F32, tag="vtr", name=f"vtr{h}_{st}_{sc}")
                                nc.tensor.transpose(vtr[:].bitcast(F32R),
                                                    vt_sb[:, sc * 128:(sc + 1) * 128], id_sb[:])
                                nc.vector.tensor_copy(v_sb[h][:, st * 4 + sc, :], vtr[:])
                        return run

                    for key in ("q", "k"):
                        for h in range(HPC):
                            bias = (bqs if key == "q" else bks)[h]
                            x_sb = aev.tile([128, ST], F32R, tag="ev", bufs=8,
                                            name=f"ev{key}{h}_{st}")
                            nc.scalar.activation(x_sb[:], acc[(key, h)][:], AF.Identity, bias=bias[:])
                            a_deferred.append(make_rope(key, h, st, ssl, x_sb))
                    for h in range(HPC):
                        vt_sb = aev.tile([128, ST], F32R, tag="evv", bufs=4, name=f"evv{h}_{st}")
                        nc.vector.tensor_scalar(vt_sb[:], acc[("v", h)][:], bvs[h][:], None, ALU.add)
                        a_deferred.append(make_vtr(h, st, vt_sb))
                while a_deferred:
                    a_deferred.pop(0)()

            # deferred non-critical DMAs (needed in B/C only)
            nc.scalar.dma_start(msk_sb[:], msk_d[:])
            for h in range(HPC):
                nc.scalar.dma_start(wo_sb[h][:], wo_d[h].bitcast(F32R))

            # ---------------- Phases B + C --------------------------------
            # greedy engine balancing for per-block elementwise passes
            load = {"dve": 0.0, "act": 0.0, "gp": 0.0}

            def pick(*opts):
                e, c = min(opts, key=lambda ec: load[ec[0]] + ec[1])
                load[e] += c
                return e

            bpools = [tc.tile_pool(name="bwork", bufs=2),
                      tc.tile_pool(name="bsmall", bufs=2),
                      tc.tile_pool(name="psS", bufs=2, space="PSUM"),
                      tc.tile_pool(name="psZ", bufs=1, space="PSUM"),
                      tc.tile_pool(name="psD", bufs=1, space="PSUM"),
                      tc.tile_pool(name="psO", bufs=1, space="PSUM")]
            with tc.tile_pool(name="cout", bufs=3) as co:
                bw, bsm, psS, psZ, psD, psO = [p.__enter__() for p in bpools]
                U32 = mybir.dt.uint32
                LOOK = 2
                pss_map = {}
                psz = {}
                psd = {}

                def emit_scores(j, kc, h):
                    jj = kc - 4 * j
                    lo = jj * 128 if 0 <= jj < 4 else 0
                    pss = psS.tile([128, ST], F32, tag="s", name=f"s{h}_{j}_{kc}")
                    nc.tensor.matmul(pss[:, lo:], krot[h][:, kc * 128:(kc + 1) * 128],
                                     qrot[h][:, j * ST + lo:(j + 1) * ST],
                                     start=True, stop=True)
                    pss_map[(j, kc, h)] = (pss, lo)

                def emit_rest(j, kc, h):
                    pss, lo = pss_map.pop((j, kc, h))
                    lsl = slice(lo, ST)
                    wdt = ST - lo
                    f = wdt / ST
                    a = bw.tile([128, ST], F32, tag="a", bufs=4, name=f"a{h}_{j}_{kc}")
                    sg = bw.tile([128, ST], F32, tag="sg", bufs=4, name=f"sg{h}_{j}_{kc}")
                    if pick(("dve", 0.76 * f), ("act", 0.80 * f)) == "dve":
                        nc.vector.tensor_scalar(a[:, lsl].bitcast(U32), pss[:, lsl].bitcast(U32),
                                                cabs_sb[:], None, ALU.bitwise_and)
                        exp_scale = C_SCALE
                        load["act"] += 0.80 * f
                        nc.scalar.activation(sg[:, lsl], pss[:, lsl], AF.Sign, bias=zero_sb[:])
                    else:
                        nc.scalar.activation(a[:, lsl], pss[:, lsl], AF.Abs,
                                             bias=zero_sb[:], scale=C_SCALE)
                        exp_scale = 1.0
                        load["dve"] += 0.76 * f
                        nc.vector.tensor_scalar(sg[:, lsl].bitcast(U32), pss[:, lsl].bitcast(U32),
                                                csgn_sb[:], cone_sb[:],
                                                ALU.bitwise_and, ALU.bitwise_or)
                    jj = kc - 4 * j
                    if 0 <= jj < 4:
                        load["gp"] += 0.25
                        nc.gpsimd.tensor_tensor(
                            a[:, lo:lo + 128], a[:, lo:lo + 128],
                            msk_sb[:, 384:512], ALU.add)
                    e2 = bw.tile([128, ST], F32R, tag="e2", bufs=4, name=f"e2{h}_{j}_{kc}")
                    load["act"] += 0.80 * f
                    nc.scalar.activation(e2[:, lsl], a[:, lsl], AF.Exp, bias=ln2_sb[:],
                                         scale=exp_scale)
                    w = bw.tile([128, ST], F32R, tag="w", bufs=6, name=f"w{h}_{j}_{kc}")
                    if pick(("dve", 0.90 * f), ("gp", 0.95 * f)) == "dve":
                        nc.vector.tensor_tensor(w[:, lsl], sg[:, lsl],
                                                e2[:, lsl].bitcast(F32), ALU.mult)
                    else:
                        nc.gpsimd.tensor_tensor(w[:, lsl], sg[:, lsl],
                                                e2[:, lsl].bitcast(F32), ALU.mult)
                    zd_deferred.append((j, kc, h, lsl, e2, w))

                def emit_zd(j, kc, h, lsl, e2, w):
                    if kc == 0:
                        psz[(j, h)] = psZ.tile([128, ST], F32, tag=f"z{h}", name=f"z{h}_{j}")
                        psd[(j, h)] = psD.tile([1, ST], F32, tag=f"d{h}", name=f"d{h}_{j}")
                    nkc_ = 4 * (j + 1)
                    nc.tensor.matmul(psd[(j, h)][:, lsl], oc_sb[:], e2[:, lsl],
                                     start=(kc == 0), stop=(kc == nkc_ - 1))
                    nc.tensor.matmul(psz[(j, h)][:, lsl], v_sb[h][:, kc, :], w[:, lsl],
                                     start=(kc == 0), stop=(kc == nkc_ - 1))

                def finalize(j, h):
                    jsl = slice(j * ST, (j + 1) * ST)
                    r32 = bsm.tile([1, ST], F32, tag="r32", name=f"r32_{h}_{j}")
                    load["dve"] += 0.75
                    nc.vector.reciprocal_approx_fast(r32[:], psd.pop((j, h))[:])
                    r_sb = bsm.tile([1, ST], F32R, tag="rsb", name=f"r{h}_{j}")
                    nc.vector.tensor_copy(r_sb[:], r32[:])
                    psrb = psO.tile([128, ST], F32, tag="o", bufs=2, name=f"rb{h}_{j}")
                    nc.tensor.matmul(psrb[:], or_sb, r_sb[:], start=True, stop=True)
                    rb_sb = bw.tile([128, ST], F32, tag="rb", name=f"rbs{h}_{j}")
                    if pick(("dve", 0.76), ("act", 0.80)) == "dve":
                        nc.vector.tensor_copy(rb_sb[:], psrb[:])
                    else:
                        nc.scalar.activation(rb_sb[:], psrb[:], AF.Copy)
                    load["dve"] += 0.87
                    nc.vector.tensor_tensor(znt[h][:, jsl], psz.pop((j, h))[:], rb_sb[:], ALU.mult)

                c_deferred = []

                def emit_C(j, pool=None):
                    pool_ = pool
                    jsl = slice(j * ST, (j + 1) * ST)
                    for mc0 in range(0, D // 128, 2):
                        c_deferred.append(make_Cpair(j, jsl, mc0, pool_))

                def make_Cpair(j, jsl, mc0, pool_):
                    def run():
                        # two pso banks interleaved: avoids same-bank accumulate stalls
                        ps_pair = [(pool_ or psO).tile([128, ST], F32, tag="o", bufs=2,
                                    name=f"o{j}_{mc0 + k}") for k in range(2)]
                        for h in range(HPC):
                            for k in range(2):
                                mc = mc0 + k
                                nc.tensor.matmul(ps_pair[k][:],
                                                 wo_sb[h][:, mc * 128:(mc + 1) * 128],
                                                 znt[h][:, jsl], start=(h == 0),
                                                 stop=(h == HPC - 1))
                        for k in range(2):
                            mc = mc0 + k
                            o_sb = co.tile([128, ST], BF16, tag="o", name=f"ev{j}_{mc}")
                            if pick(("dve", 0.76), ("act", 0.80)) == "dve":
                                nc.vector.tensor_copy(o_sb[:], ps_pair[k][:])
                            else:
                                nc.scalar.activation(o_sb[:], ps_pair[k][:], AF.Copy)
                            nc.sync.dma_start(out_d[mc * 128:(mc + 1) * 128, jsl], o_sb[:])
                    return run

                blocks = [(j, kc, h) for j in range(NST)
                          for kc in range(4 * (j + 1)) for h in range(HPC)]
                zd_deferred = []
                ZLAG = 2
                done_in_j = {j: 0 for j in range(NST)}

                def pump_zd(limit=1):
                    while len(zd_deferred) > ZLAG if limit is None else \
                            (limit > 0 and len(zd_deferred) > ZLAG):
                        args = zd_deferred.pop(0)
                        emit_zd(*args)
                        after_rest(args[0])
                        if limit is not None:
                            limit -= 1

                fin_pending = []   # [j, ticks, stage]

                def after_rest(j):
                    done_in_j[j] += 1
                    if done_in_j[j] == 2 * 4 * (j + 1):
                        fin_pending.append([j, 0, 0])

                def fin_tick(force=False):
                    for e in fin_pending:
                        e[1] += 1
                    if not fin_pending:
                        return
                    j_, t_, stage = fin_pending[0]
                    if stage == 0 and (force or t_ >= 1):
                        finalize(j_, 0)
                        fin_pending[0][2] = 1
                    elif stage == 1 and (force or t_ >= 4):
                        fin_pending.pop(0)
                        finalize(j_, 1)
                        emit_C(j_)

                for i, (j, kc, h) in enumerate(blocks):
                    emit_scores(j, kc, h)
                    if i >= LOOK:
                        jj_, kc_, h_ = blocks[i - LOOK]
                        emit_rest(jj_, kc_, h_)
                        pump_zd(1)
                    fin_tick()
                    if c_deferred:
                        c_deferred.pop(0)()
                for (j, kc, h) in blocks[len(blocks) - LOOK:]:
                    emit_rest(j, kc, h)
                    pump_zd(1)
                while zd_deferred:
                    args = zd_deferred.pop(0)
                    emit_zd(*args)
                    after_rest(args[0])
                    fin_tick()
                while fin_pending:
                    fin_tick(force=True)
                    fin_tick(force=True)
                while c_deferred:
                    c_deferred.pop(0)()
                for p in reversed(bpools):
                    p.__exit__(None, None, None)
    nc.compile()
    return nc


def _host_constants():
    inv = 1.0 / (10000.0 ** (np.arange(0, DH, 2, dtype=np.float32) / DH))
    t = np.arange(S, dtype=np.float32)
    fr = t[:, None] * inv[None, :]                       # [S, DH/2]
    cosT = np.repeat(np.cos(fr).astype(np.float32).T, 2, axis=0)  # [DH, S]
    sinT = np.repeat(np.sin(fr).astype(np.float32).T, 2, axis=0)

    # sliding causal mask: msk[k, c] = 0 if k <= c - 384 else MASK_NEG
    kk = np.arange(128)[:, None]
    cc = np.arange(896)[None, :]
    msk = np.where(kk <= cc - 384, 0.0, MASK_NEG).astype(np.float32)

    # pt = P.T with P@x the rotate-half shuffle: (P x)[2i] = -x[2i+1], (P x)[2i+1] = x[2i]
    pt = np.zeros((128, 128), dtype=np.float32)
    i = np.arange(0, 128, 2)
    pt[i + 1, i] = -1.0
    pt[i, i + 1] = 1.0

    ident = np.eye(128, dtype=np.float32)
    onescol = np.ones((128, 1), dtype=np.float32)
    onesrow = np.ones((2, 128), dtype=np.float32)  # rb = 1/D2; row0 used; padded for cache key
    return cosT, sinT, msk, pt, ident, onescol, onesrow


def _run(inputs, trace=False, trace_kwargs=None):
    from concourse.bass_utils import run_bass_kernel_spmd

    if "nc" not in _CACHE:
        _CACHE["nc"] = _build_program()
    nc = _CACHE["nc"]

    resid_pre = np.asarray(inputs["resid_pre"], dtype=np.float32)
    W_Q = np.asarray(inputs["W_Q"], dtype=np.float32)
    W_K = np.asarray(inputs["W_K"], dtype=np.float32)
    W_V = np.asarray(inputs["W_V"], dtype=np.float32)
    W_O = np.asarray(inputs["W_O"], dtype=np.float32)
    b_Q = np.asarray(inputs["b_Q"], dtype=np.float32)
    b_K = np.asarray(inputs["b_K"], dtype=np.float32)
    b_V = np.asarray(inputs["b_V"], dtype=np.float32)
    b_O = np.asarray(inputs["b_O"], dtype=np.float32)

    xt = np.ascontiguousarray(resid_pre[0].T)
    cosT, sinT, msk, pt, ident, onescol, onesrow = _host_constants()

    in_maps = []
    for c in range(NC):
        hs = slice(c * HPC, (c + 1) * HPC)
        wl = np.empty((NDC, 6, 128, DH), dtype=np.float32)
        for dc in range(NDC):
            sl = slice(dc * 128, (dc + 1) * 128)
            wl[dc, 0] = W_Q[c * HPC + 0][sl]
            wl[dc, 1] = W_Q[c * HPC + 1][sl]
            wl[dc, 2] = W_K[c * HPC + 0][sl]
            wl[dc, 3] = W_K[c * HPC + 1][sl]
            wl[dc, 4] = W_V[c * HPC + 0][sl]
            wl[dc, 5] = W_V[c * HPC + 1][sl]
        in_maps.append({
            "xt": xt,
            "wall": wl,
            "wo": np.ascontiguousarray(W_O[hs]),
            "bq": np.ascontiguousarray(b_Q[hs][:, :, None]),
            "bk": np.ascontiguousarray(b_K[hs][:, :, None]),
            "bv": np.ascontiguousarray(b_V[hs][:, :, None]),
            "cost": cosT, "sint": sinT, "maskneg": msk, "pt": pt,
            "ident": ident, "onescol": onescol, "onesrow": onesrow,
        })

    kw = dict(trace_kwargs or {})
    last_err = None
    for attempt in range(3):
        try:
            res = run_bass_kernel_spmd(nc, in_maps, list(range(NC)), trace=trace, **kw)
            break
        except Exception as e:  # transient NRT_EXEC_UNIT_UNRECOVERABLE wedges clear on retry
            last_err = e
            if attempt == 2 or "UNRECOVERABLE" not in str(e).upper() and "UNAVAILABLE" not in str(e).upper():
                raise
            import time
            time.sleep(3.0)
    else:
        raise last_err

    acc = np.zeros((D, S), dtype=np.float32)
    for c in range(NC):
        acc += np.asarray(res.results[c]["outt"]).astype(np.float32)
    out = acc.T + b_O[None, :]
    return out.reshape(1, S, D).astype(np.float32), res


def kernel(**inputs) -> np.ndarray:
    out, _ = _run(inputs, trace=False)
    return out


# revision 39
# speedup vs baseline: 1.5151x; 1.0205x over previous
"""Trainium2 Bass kernel for nn_Attention_10015863734775.

Multi-head causal attention (16 heads, d_model 2048, d_head 128, seq 2048,
batch 1) with llama-style interleaved RoPE and a signed-softmax:
    attn_w = sign(s) * exp(|s| - max|s|);  attn = attn_w / (sum|attn_w| + 1e-6)
The max-subtraction cancels in the normalization (scores are O(5), exp is
safe in fp32), so the device computes attn = sign(s)exp(|s|) / sum exp(|s|).

Sharding: 2 heads per NeuronCore (8 cores). Each core receives the full
transposed residual X^T plus its head slices of W_Q/K/V/W_O and computes a
partial output projection outT_c[m, s] (bf16); the host sums the 8 partials
in f32, transposes, and adds b_O (exact: b_O enters after all nonlinearities).

Device layouts are all [feature, seq] ("T" layouts) so that:
  - scoresT[k, q] blocks come straight from matmul(lhsT=kT chunk, rhs=qT)
  - the z matmul needs no transposes (V is transposed once via the PE)
  - the signed-softmax k-sum is a ones-vector matmul on the PE
Causal masking skips fully-masked blocks; diagonal blocks add a -1e5 mask
to |s| before exp (exp underflows to exactly 0).
All matmuls run as float32r (~tf32 operand rounding, full fp32 accumulate).

Deltas over the original baseline (each measured):
  - reciprocal -> reciprocal_approx_fast (~5x) + f32r rounding copy
  - output staging + DMA in bf16 (halves output traffic; host sums in f32)
  - causal-mask adds on GPSIMD (SBUF-only op; frees DVE/ACT cycles)
"""

import math

import numpy as np

S = 2048          # sequence length
D = 2048          # d_model
DH = 128          # d_head
NH = 16           # total heads
NC = 8            # neuron cores
HPC = NH // NC    # heads per core (2)
ST = 512          # seq tile (matmul free dim / one PSUM bank)
NST = S // ST     # 4 seq tiles
NDC = D // 128    # 16 contraction chunks
NKC = S // 128    # 16 k chunks
C_SCALE = 1.0 / math.sqrt(float(DH))
LN2 = math.log(2.0)
MASK_NEG = -1.0e5

_CACHE = {}


def _build_program():
    import concourse.tile as tile
    from concourse import bacc, mybir

    F32 = mybir.dt.float32
    F32R = mybir.dt.float32r
    BF16 = mybir.dt.bfloat16
    AF = mybir.ActivationFunctionType
    ALU = mybir.AluOpType

    nc = bacc.Bacc("TRN2", target_bir_lowering=False, debug=False, num_devices=NC)

    xt_d = nc.dram_tensor("xt", [D, S], F32, kind="ExternalInput").ap()
    wall_d = nc.dram_tensor("wall", [NDC, 6, 128, DH], F32, kind="ExternalInput").ap()
    wo_d = nc.dram_tensor("wo", [HPC, DH, D], F32, kind="ExternalInput").ap()
    bq_d = nc.dram_tensor("bq", [HPC, DH, 1], F32, kind="ExternalInput").ap()
    bk_d = nc.dram_tensor("bk", [HPC, DH, 1], F32, kind="ExternalInput").ap()
    bv_d = nc.dram_tensor("bv", [HPC, DH, 1], F32, kind="ExternalInput").ap()
    cos_d = nc.dram_tensor("cost", [DH, S], F32, kind="ExternalInput").ap()
    sin_d = nc.dram_tensor("sint", [DH, S], F32, kind="ExternalInput").ap()
    msk_d = nc.dram_tensor("maskneg", [128, 896], F32, kind="ExternalInput").ap()
    pt_d = nc.dram_tensor("pt", [128, 128], F32, kind="ExternalInput").ap()
    id_d = nc.dram_tensor("ident", [128, 128], F32, kind="ExternalInput").ap()
    oc_d = nc.dram_tensor("onescol", [128, 1], F32, kind="ExternalInput").ap()
    or_d = nc.dram_tensor("onesrow", [2, 128], F32, kind="ExternalInput").ap()
    out_d = nc.dram_tensor("outt", [D, S], BF16, kind="ExternalOutput").ap()

    with tile.TileContext(nc) as tc:
        with tc.tile_pool(name="persist", bufs=1) as pp:
            # persistent SBUF
            wo_sb = []
            bqs, bks, bvs = [], [], []
            qrot, krot, v_sb, znt = [], [], [], []
            for h in range(HPC):
                wo_sb.append(pp.tile([DH, D], F32R, tag=f"wo{h}", name=f"wo{h}"))
                for lst, dd, nm in ((bqs, bq_d, "bq"), (bks, bk_d, "bk"), (bvs, bv_d, "bv")):
                    bt = pp.tile([DH, 1], F32, tag=f"{nm}{h}")
                    nc.scalar.dma_start(bt[:], dd[h])
                    lst.append(bt)
                qrot.append(pp.tile([DH, S], F32R, tag=f"qrot{h}", name=f"qrot{h}"))
                krot.append(pp.tile([DH, S], F32R, tag=f"krot{h}", name=f"krot{h}"))
                v_sb.append(pp.tile([128, NKC, DH], F32R, tag=f"v{h}", name=f"v{h}"))
                znt.append(pp.tile([DH, S], F32R, tag=f"znt{h}", name=f"znt{h}"))
            msk_sb = pp.tile([128, 896], F32, tag="msk")
            pt_sb = pp.tile([128, 128], F32R, tag="pt")
            nc.scalar.dma_start(pt_sb[:], pt_d[:].bitcast(F32R))
            id_sb = pp.tile([128, 128], F32R, tag="ident")
            nc.scalar.dma_start(id_sb[:], id_d[:].bitcast(F32R))
            oc_sb = pp.tile([128, 1], F32R, tag="onescol")
            nc.scalar.dma_start(oc_sb[:], oc_d[:].bitcast(F32R))
            or_sb_full = pp.tile([2, 128], F32R, tag="onesrow")
            nc.scalar.dma_start(or_sb_full[:], or_d[:].bitcast(F32R))
            or_sb = or_sb_full[0:1, :]
            cabs_sb = pp.tile([128, 1], mybir.dt.uint32, tag="cabs")
            nc.vector.memset(cabs_sb[:], 0x7FFFFFFF)
            csgn_sb = pp.tile([128, 1], mybir.dt.uint32, tag="csgn")
            nc.vector.memset(csgn_sb[:], 0x80000000)
            cone_sb = pp.tile([128, 1], mybir.dt.uint32, tag="cone")
            nc.vector.memset(cone_sb[:], 0x3F800000)
            ln2_sb = pp.tile([128, 1], F32, tag="ln2")
            nc.vector.memset(ln2_sb[:], LN2)
            zero_sb = pp.tile([128, 1], F32, tag="zero")
            nc.vector.memset(zero_sb[:], 0.0)
            import os as _os
            _nonce = float(int(_os.environ.get("KBUILD_NONCE", "0")))
            nonce_sb = pp.tile([128, 1], F32, tag="nonce")
            nc.vector.memset(nonce_sb[:], _nonce)

            # ---------------- Phase A: projections + RoPE + V transpose ----
            with tc.tile_pool(name="aphase", bufs=1) as ap_, \
                 tc.tile_pool(name="axt", bufs=6) as axt, \
                 tc.tile_pool(name="aev", bufs=6) as aev, \
                 tc.tile_pool(name="arope", bufs=4) as arp, \
                 tc.tile_pool(name="psA", bufs=1, space="PSUM") as psA, \
                 tc.tile_pool(name="psShuf", bufs=1, space="PSUM") as psSh, \
                 tc.tile_pool(name="psVtr", bufs=1, space="PSUM") as psVt:
                wall_sb = ap_.tile([128, NDC, 6, DH], F32R, tag="wall", name="wall_sb")
                widx = {("q", 0): 0, ("q", 1): 1, ("k", 0): 2, ("k", 1): 3,
                        ("v", 0): 4, ("v", 1): 5}
                cos_sb = ap_.tile([DH, S], F32, tag="cos")
                sin_sb = ap_.tile([DH, S], F32, tag="sin")

                a_deferred = []
                for st in range(NST):
                    ssl = slice(st * ST, (st + 1) * ST)
                    acc = {}
                    for key in ("q", "k", "v"):
                        for h in range(HPC):
                            acc[(key, h)] = psA.tile([128, ST], F32, tag=f"acc{key}{h}", name=f"acc{key}{h}")
                    for dc in range(NDC):
                        if st == 0:
                            nc.gpsimd.dma_start(
                                wall_sb[:, dc, :, :],
                                wall_d[dc].rearrange("i p e -> p i e").bitcast(F32R))
                        xt_t = axt.tile([128, ST], F32R, tag="xt")
                        nc.sync.dma_start(
                            xt_t[:],
                            xt_d[dc * 128:(dc + 1) * 128, ssl].bitcast(F32R),
                        )
                        if st == 0 and dc == 4:
                            nc.scalar.dma_start(cos_sb[:], cos_d[:])
                            nc.scalar.dma_start(sin_sb[:], sin_d[:])
                        for key in ("q", "k", "v"):
                            for h in range(HPC):
                                nc.tensor.matmul(
                                    acc[(key, h)][:], wall_sb[:, dc, widx[(key, h)], :], xt_t[:],
                                    start=(dc == 0), stop=(dc == NDC - 1),
                                )
                        if a_deferred and dc % 2 == 1:
                            a_deferred.pop(0)()
                    # evictions free the acc slots now; the rope/transpose PE work is
                    # deferred into the next st's dc loop (A-phase software pipeline)
                    def make_rope(key, h, st, ssl, x_sb):
                        def run():
                            dst = (qrot if key == "q" else krot)[h]
                            shuf = psSh.tile([128, ST], F32, tag="shuf", name=f"sh{key}{h}_{st}")
                            nc.tensor.matmul(shuf[:], pt_sb[:], x_sb[:],
                                             start=True, stop=True)
                            t1 = arp.tile([128, ST], F32, tag="t1", name=f"t1{key}{h}_{st}")
                            nc.gpsimd.tensor_tensor(t1[:], x_sb[:].bitcast(F32), cos_sb[:, ssl], ALU.mult)
                            t2 = arp.tile([128, ST], F32, tag="t2", name=f"t2{key}{h}_{st}")
                            nc.vector.tensor_tensor(t2[:], shuf[:], sin_sb[:, ssl], ALU.mult)
                            nc.vector.tensor_tensor(dst[:, ssl], t1[:], t2[:], ALU.add)
                        return run

                    def make_vtr(h, st, vt_sb):
                        def run():
                            for sc in range(ST // 128):
                                vtr = psVt.tile([128, 128], F32, tag="vtr", name=f"vtr{h}_{st}_{sc}")
                                nc.tensor.transpose(vtr[:].bitcast(F32R),
                                                    vt_sb[:, sc * 128:(sc + 1) * 128], id_sb[:])
                                nc.scalar.activation(v_sb[h][:, st * 4 + sc, :], vtr[:],
                                                     AF.Copy)
                        return run

                    for key in ("q", "k"):
                        for h in range(HPC):
                            bias = (bqs if key == "q" else bks)[h]
                            x_sb = aev.tile([128, ST], F32R, tag="ev", bufs=8,
                                            name=f"ev{key}{h}_{st}")
                            nc.scalar.activation(x_sb[:], acc[(key, h)][:], AF.Identity, bias=bias[:])
                            a_deferred.append(make_rope(key, h, st, ssl, x_sb))
                    for h in range(HPC):
                        vt_sb = aev.tile([128, ST], F32R, tag="evv", bufs=4, name=f"evv{h}_{st}")
                        nc.scalar.activation(vt_sb[:], acc[("v", h)][:], AF.Identity,
                                             bias=bvs[h][:])
                        a_deferred.append(make_vtr(h, st, vt_sb))
                while a_deferred:
                    a_deferred.pop(0)()

            # deferred non-critical DMAs (needed in B/C only)
            nc.scalar.dma_start(msk_sb[:], msk_d[:])
            for h in range(HPC):
                nc.scalar.dma_start(wo_sb[h][:], wo_d[h].bitcast(F32R))

            # ---------------- Phases B + C --------------------------------
            # greedy engine balancing for per-block elementwise passes
            load = {"dve": 0.0, "act": 0.0, "gp": 0.0}

            def pick(*opts):
                e, c = min(opts, key=lambda ec: load[ec[0]] + ec[1])
                load[e] += c
                return e

            bpools = [tc.tile_pool(name="bwork", bufs=2),
                      tc.tile_pool(name="bsmall", bufs=2),
                      tc.tile_pool(name="psS", bufs=2, space="PSUM"),
                      tc.tile_pool(name="psZ", bufs=1, space="PSUM"),
                      tc.tile_pool(name="psD", bufs=1, space="PSUM"),
                      tc.tile_pool(name="psO", bufs=1, space="PSUM")]
            with tc.tile_pool(name="cout", bufs=3) as co:
                bw, bsm, psS, psZ, psD, psO = [p.__enter__() for p in bpools]
                U32 = mybir.dt.uint32
                LOOK = 2
                pss_map = {}
                psz = {}
                psd = {}

                def emit_scores(j, kc, h):
                    jj = kc - 4 * j
                    lo = jj * 128 if 0 <= jj < 4 else 0
                    pss = psS.tile([128, ST], F32, tag="s", name=f"s{h}_{j}_{kc}")
                    nc.tensor.matmul(pss[:, lo:], krot[h][:, kc * 128:(kc + 1) * 128],
                                     qrot[h][:, j * ST + lo:(j + 1) * ST],
                                     start=True, stop=True)
                    pss_map[(j, kc, h)] = (pss, lo)

                def emit_rest(j, kc, h):
                    pss, lo = pss_map.pop((j, kc, h))
                    lsl = slice(lo, ST)
                    wdt = ST - lo
                    f = wdt / ST
                    a = bw.tile([128, ST], F32, tag="a", bufs=4, name=f"a{h}_{j}_{kc}")
                    sg = bw.tile([128, ST], F32, tag="sg", bufs=4, name=f"sg{h}_{j}_{kc}")
                    if pick(("dve", 0.76 * f), ("act", 0.80 * f)) == "dve":
                        nc.vector.tensor_scalar(a[:, lsl].bitcast(U32), pss[:, lsl].bitcast(U32),
                                                cabs_sb[:], None, ALU.bitwise_and)
                        exp_scale = C_SCALE
                        load["act"] += 0.80 * f
                        nc.scalar.activation(sg[:, lsl], pss[:, lsl], AF.Sign, bias=zero_sb[:])
                    else:
                        nc.scalar.activation(a[:, lsl], pss[:, lsl], AF.Abs,
                                             bias=zero_sb[:], scale=C_SCALE)
                        exp_scale = 1.0
                        load["dve"] += 0.76 * f
                        nc.vector.tensor_scalar(sg[:, lsl].bitcast(U32), pss[:, lsl].bitcast(U32),
                                                csgn_sb[:], cone_sb[:],
                                                ALU.bitwise_and, ALU.bitwise_or)
                    jj = kc - 4 * j
                    if 0 <= jj < 4:
                        load["gp"] += 0.25
                        nc.gpsimd.tensor_tensor(
                            a[:, lo:lo + 128], a[:, lo:lo + 128],
                            msk_sb[:, 384:512], ALU.add)
                    e2 = bw.tile([128, ST], F32R, tag="e2", bufs=4, name=f"e2{h}_{j}_{kc}")
                    load["act"] += 0.80 * f
                    nc.scalar.activation(e2[:, lsl], a[:, lsl], AF.Exp, bias=ln2_sb[:],
                                         scale=exp_scale)
                    w = bw.tile([128, ST], F32R, tag="w", bufs=6, name=f"w{h}_{j}_{kc}")
                    if pick(("dve", 0.90 * f), ("gp", 0.95 * f)) == "dve":
                        nc.vector.tensor_tensor(w[:, lsl], sg[:, lsl],
                                                e2[:, lsl].bitcast(F32), ALU.mult)
                    else:
                        nc.gpsimd.tensor_tensor(w[:, lsl], sg[:, lsl],
                                                e2[:, lsl].bitcast(F32), ALU.mult)
                    zd_deferred.append((j, kc, h, lsl, e2, w))

                def emit_zd(j, kc, h, lsl, e2, w):
                    if kc == 0:
                        psz[(j, h)] = psZ.tile([128, ST], F32, tag=f"z{h}", name=f"z{h}_{j}")
                        psd[(j, h)] = psD.tile([1, ST], F32, tag=f"d{h}", name=f"d{h}_{j}")
                    nkc_ = 4 * (j + 1)
                    nc.tensor.matmul(psd[(j, h)][:, lsl], oc_sb[:], e2[:, lsl],
                                     start=(kc == 0), stop=(kc == nkc_ - 1))
                    nc.tensor.matmul(psz[(j, h)][:, lsl], v_sb[h][:, kc, :], w[:, lsl],
                                     start=(kc == 0), stop=(kc == nkc_ - 1))

                def finalize(j, h):
                    jsl = slice(j * ST, (j + 1) * ST)
                    r32 = bsm.tile([1, ST], F32, tag="r32", name=f"r32_{h}_{j}")
                    load["dve"] += 0.75
                    nc.vector.reciprocal_approx_fast(r32[:], psd.pop((j, h))[:])
                    r_sb = bsm.tile([1, ST], F32R, tag="rsb", name=f"r{h}_{j}")
                    nc.vector.tensor_copy(r_sb[:], r32[:])
                    psrb = psO.tile([128, ST], F32, tag="o", bufs=2, name=f"rb{h}_{j}")
                    nc.tensor.matmul(psrb[:], or_sb, r_sb[:], start=True, stop=True)
                    rb_sb = bw.tile([128, ST], F32, tag="rb", name=f"rbs{h}_{j}")
                    if pick(("dve", 0.76), ("act", 0.80)) == "dve":
                        nc.vector.tensor_copy(rb_sb[:], psrb[:])
                    else:
                        nc.scalar.activation(rb_sb[:], psrb[:], AF.Copy)
                    load["dve"] += 0.87
                    nc.vector.tensor_tensor(znt[h][:, jsl], psz.pop((j, h))[:], rb_sb[:], ALU.mult)

                c_deferred = []

                def emit_C(j, pool=None):
                    pool_ = pool
                    jsl = slice(j * ST, (j + 1) * ST)
                    for mc0 in range(0, D // 128, 2):
                        c_deferred.append(make_Cpair(j, jsl, mc0, pool_))

                def make_Cpair(j, jsl, mc0, pool_):
                    def run():
                        # two pso banks interleaved: avoids same-bank accumulate stalls
                        ps_pair = [(pool_ or psO).tile([128, ST], F32, tag="o", bufs=2,
                                    name=f"o{j}_{mc0 + k}") for k in range(2)]
                        for h in range(HPC):
                            for k in range(2):
                                mc = mc0 + k
                                nc.tensor.matmul(ps_pair[k][:],
                                                 wo_sb[h][:, mc * 128:(mc + 1) * 128],
                                                 znt[h][:, jsl], start=(h == 0),
                                                 stop=(h == HPC - 1))
                        for k in range(2):
                            mc = mc0 + k
                            o_sb = co.tile([128, ST], BF16, tag="o", name=f"ev{j}_{mc}")
                            if pick(("dve", 0.76), ("act", 0.80)) == "dve":
                                nc.vector.tensor_copy(o_sb[:], ps_pair[k][:])
                            else:
                                nc.scalar.activation(o_sb[:], ps_pair[k][:], AF.Copy)
                            nc.sync.dma_start(out_d[mc * 128:(mc + 1) * 128, jsl], o_sb[:])
                    return run

                blocks = [(j, kc, h) for j in range(NST)
                          for kc in range(4 * (j + 1)) for h in range(HPC)]
                zd_deferred = []
                ZLAG = 2
                done_in_j = {j: 0 for j in range(NST)}

                def pump_zd(limit=1):
                    while len(zd_deferred) > ZLAG if limit is None else \
                            (limit > 0 and len(zd_deferred) > ZLAG):
                        args = zd_deferred.pop(0)
                        emit_zd(*args)
                        after_rest(args[0])
                        if limit is not None:
                            limit -= 1

                fin_pending = []   # [j, ticks, stage]

                def after_rest(j):
                    done_in_j[j] += 1
                    if done_in_j[j] == 2 * 4 * (j + 1):
                        fin_pending.append([j, 0, 0])

                def fin_tick(force=False):
                    for e in fin_pending:
                        e[1] += 1
                    if not fin_pending:
                        return
                    j_, t_, stage = fin_pending[0]
                    if stage == 0 and (force or t_ >= 1):
                        finalize(j_, 0)
                        fin_pending[0][2] = 1
                    elif stage == 1 and (force or t_ >= 4):
                        fin_pending.pop(0)
                        finalize(j_, 1)
                        emit_C(j_)

                for i, (j, kc, h) in enumerate(blocks):
                    emit_scores(j, kc, h)
                    if i >= LOOK:
                        jj_, kc_, h_ = blocks[i - LOOK]
                        emit_rest(jj_, kc_, h_)
                        pump_zd(1)
                    fin_tick()
                    if c_deferred:
                        c_deferred.pop(0)()
                for (j, kc, h) in blocks[len(blocks) - LOOK:]:
                    emit_rest(j, kc, h)
                    pump_zd(1)
                while zd_deferred:
                    args = zd_deferred.pop(0)
                    emit_zd(*args)
                    after_rest(args[0])
                    fin_tick()
                while fin_pending:
                    fin_tick(force=True)
                    fin_tick(force=True)
                while c_deferred:
                    c_deferred.pop(0)()
                for p in reversed(bpools):
                    p.__exit__(None, None, None)
    nc.compile()
    return nc


def _host_constants():
    inv = 1.0 / (10000.0 ** (np.arange(0, DH, 2, dtype=np.float32) / DH))
    t = np.arange(S, dtype=np.float32)
    fr = t[:, None] * inv[None, :]                       # [S, DH/2]
    cosT = np.repeat(np.cos(fr).astype(np.float32).T, 2, axis=0)  # [DH, S]
    sinT = np.repeat(np.sin(fr).astype(np.float32).T, 2, axis=0)

    # sliding causal mask: msk[k, c] = 0 if k <= c - 384 else MASK_NEG
    kk = np.arange(128)[:, None]
    cc = np.arange(896)[None, :]
    msk = np.where(kk <= cc - 384, 0.0, MASK_NEG).astype(np.float32)

    # pt = P.T with P@x the rotate-half shuffle: (P x)[2i] = -x[2i+1], (P x)[2i+1] = x[2i]
    pt = np.zeros((128, 128), dtype=np.float32)
    i = np.arange(0, 128, 2)
    pt[i + 1, i] = -1.0
    pt[i, i + 1] = 1.0

    ident = np.eye(128, dtype=np.float32)
    onescol = np.ones((128, 1), dtype=np.float32)
    onesrow = np.ones((2, 128), dtype=np.float32)  # rb = 1/D2; row0 used; padded for cache key
    return cosT, sinT, msk, pt, ident, onescol, onesrow


def _run(inputs, trace=False, trace_kwargs=None):
    from concourse.bass_utils import run_bass_kernel_spmd

    if "nc" not in _CACHE:
        _CACHE["nc"] = _build_program()
    nc = _CACHE["nc"]

    resid_pre = np.asarray(inputs["resid_pre"], dtype=np.float32)
    W_Q = np.asarray(inputs["W_Q"], dtype=np.float32)
    W_K = np.asarray(inputs["W_K"], dtype=np.float32)
    W_V = np.asarray(inputs["W_V"], dtype=np.float32)
    W_O = np.asarray(inputs["W_O"], dtype=np.float32)
    b_Q = np.asarray(inputs["b_Q"], dtype=np.float32)
    b_K = np.asarray(inputs["b_K"], dtype=np.float32)
    b_V = np.asarray(inputs["b_V"], dtype=np.float32)
    b_O = np.asarray(inputs["b_O"], dtype=np.float32)

    xt = np.ascontiguousarray(resid_pre[0].T)
    cosT, sinT, msk, pt, ident, onescol, onesrow = _host_constants()

    in_maps = []
    for c in range(NC):
        hs = slice(c * HPC, (c + 1) * HPC)
        wl = np.empty((NDC, 6, 128, DH), dtype=np.float32)
        for dc in range(NDC):
            sl = slice(dc * 128, (dc + 1) * 128)
            wl[dc, 0] = W_Q[c * HPC + 0][sl]
            wl[dc, 1] = W_Q[c * HPC + 1][sl]
            wl[dc, 2] = W_K[c * HPC + 0][sl]
            wl[dc, 3] = W_K[c * HPC + 1][sl]
            wl[dc, 4] = W_V[c * HPC + 0][sl]
            wl[dc, 5] = W_V[c * HPC + 1][sl]
        in_maps.append({
            "xt": xt,
            "wall": wl,
            "wo": np.ascontiguousarray(W_O[hs]),
            "bq": np.ascontiguousarray(b_Q[hs][:, :, None]),
            "bk": np.ascontiguousarray(b_K[hs][:, :, None]),
            "bv": np.ascontiguousarray(b_V[hs][:, :, None]),
            "cost": cosT, "sint": sinT, "maskneg": msk, "pt": pt,
            "ident": ident, "onescol": onescol, "onesrow": onesrow,
        })

    kw = dict(trace_kwargs or {})
    last_err = None
    for attempt in range(3):
        try:
            res = run_bass_kernel_spmd(nc, in_maps, list(range(NC)), trace=trace, **kw)
            break
        except Exception as e:  # transient NRT_EXEC_UNIT_UNRECOVERABLE wedges clear on retry
            last_err = e
            if attempt == 2 or "UNRECOVERABLE" not in str(e).upper() and "UNAVAILABLE" not in str(e).upper():
                raise
            import time
            time.sleep(3.0)
    else:
        raise last_err

    acc = np.zeros((D, S), dtype=np.float32)
    for c in range(NC):
        acc += np.asarray(res.results[c]["outt"]).astype(np.float32)
    out = acc.T + b_O[None, :]
    return out.reshape(1, S, D).astype(np.float32), res


def kernel(**inputs) -> np.ndarray:
    out, _ = _run(inputs, trace=False)
    return out
